# revision 30
# baseline (speedup 1.0000x reference)
"""AttentionFuserV3 Trainium2 kernel: 8-core pure data parallel over batch.

Reference computation per batch item x_b [L=1024, D=512]:
  stage1: q = x W1^T; S = q x^T; A = softmax(S); mix = A x;
          h = tanh([mix, q] Wo1^T); h = h / max(||h||_2, eps)     (per row)
  stage2: c = [h, x]; q2 = c W2^T; S2 = q2 c^T; A2 = softmax(S2);
          mix2 = A2 c; o = [mix2, q2] Wo2^T; emb = mean_l(o)

Layout strategy ("T-space"): all big tensors are kept transposed in SBUF
(feature dim on partitions, sequence dim L on the free axis) so every
matmul contraction lands on the partition axis without on-device
transposes of the attention matrix.  Softmax runs without max-subtraction
(|scores| < ~70, exp stays in f32 range); softmax denominators are
accumulated pre-broadcast with a ones-matrix lhsT (M=128 costs the same
as M=1 on the PE) and inverted full-width with the fast approximate
reciprocal.

Stage-2 exploits linearity of the final mean:
  emb = mean_l(out2) = (1/L) [sum_l mix2 ; sum_l q2] Wo2^T
  sum_l mix2 = c2^T r   with   r[m] = sum_l exp2[m,l] / denom2[l]
so mix2 and out2 are never materialized per position; r is reduced on
DVE from the transposed exp2 tile, broadcast back with rank-1 matmuls,
and the Wo2 projection happens once for all batch items at the end.

The per-item tail (r extraction, broadcast, weighted row-sums) is
software-pipelined: its PE/DVE work is emitted interleaved into the NEXT
item's stage-1 stream so the PE never waits on the serial DVE chain.

Matmuls run in float32r (full PE speed at N=512); the attention
probabilities and the mix lhsT (x natural) are bf16.
"""

import sys

sys.path.insert(0, "/opt/trn_rl_repo")

import numpy as np

N_GLOBAL, L, D = 32, 1024, 512
NCORES = 8
B = N_GLOBAL // NCORES          # 4 batch items per core
P = 128
LC = 512                        # l-chunk (matmul moving free dim)
NLC = L // LC                   # 2
DT = D // P                     # 4
LT = L // P                     # 8
D2T = 2 * D // P                # 8
C2T = 4 * D // P                # 16

_CACHE = {}


def _build_nc(xt_bufs=2):
    import concourse.bass as bass  # noqa: F401
    import concourse.mybir as mybir
    import concourse.tile as tile
    from concourse import bacc

    f32 = mybir.dt.float32
    f32r = mybir.dt.float32r
    bf16 = mybir.dt.bfloat16
    AF = mybir.ActivationFunctionType
    ALU = mybir.AluOpType
    AXX = mybir.AxisListType.X

    nc = bacc.Bacc("TRN2", target_bir_lowering=False, debug=False,
                   num_devices=NCORES)

    x_ext = nc.declare_dram_parameter("x", [B, L, D], bf16, isOutput=False)
    xT_ext = nc.declare_dram_parameter("xT", [B, D, L], f32r, isOutput=False)
    w1t_ext = nc.declare_dram_parameter("w1t", [D, D], f32r, isOutput=False)
    wo1t_ext = nc.declare_dram_parameter("wo1t", [2 * D, D], f32r, isOutput=False)
    w2t_ext = nc.declare_dram_parameter("w2t", [2 * D, 2 * D], f32r, isOutput=False)
    wo2t_ext = nc.declare_dram_parameter("wo2t", [4 * D, D], bf16, isOutput=False)
    # Constants shipped from host: walrus's ISA check rejects memset/iota
    # writes into float32r tiles, but DMA from an f32r DRAM param is fine.
    onm_ext = nc.declare_dram_parameter("onesm", [P, P], bf16, isOutput=False)
    onr_ext = nc.declare_dram_parameter("onesr", [1, P], bf16, isOutput=False)
    out_ext = nc.declare_dram_parameter("out", [B, D], f32, isOutput=True)

    import time as _time
    _t0 = _time.time()
    with tile.TileContext(nc) as tc:
        with tc.tile_pool(name="wp", bufs=1) as wp, \
             tc.tile_pool(name="cp", bufs=1) as cp, \
             tc.tile_pool(name="xtp", bufs=xt_bufs) as xtp, \
             tc.tile_pool(name="xp", bufs=2) as xp, \
             tc.tile_pool(name="hp", bufs=2) as hp, \
             tc.tile_pool(name="tp", bufs=1) as tp, \
             tc.tile_pool(name="vp", bufs=2) as vp, \
             tc.tile_pool(name="ep", bufs=1) as ep, \
             tc.tile_pool(name="ps", bufs=8, space="PSUM") as pp:

            # ---- w1t gates ph1 of item 0: DMA it first (chunks of it
            # interleave with the xT chunks inside the b==0 iteration)
            w1t_s = wp.tile([P, DT, D], f32r, tag="w1t")
            onesm_s = cp.tile([P, P], bf16, tag="onesm")
            nc.sync.dma_start(out=onesm_s, in_=onm_ext[:, :])
            onesr_s = cp.tile([1, P], bf16, tag="onesr")
            nc.sync.dma_start(out=onesr_s, in_=onr_ext[:, :])

            wo1t_s = wp.tile([P, D2T, D], f32r, tag="wo1t")
            w2t_s = wp.tile([P, D2T, 2 * D], f32r, tag="w2t")
            wo2t_s = wp.tile([P, C2T, D], bf16, tag="wo2t")

            # meanvec columns for the deferred Wo2 projection:
            # c-chunks 0..3 = sum_l mix2 (h part), 4..7 (x part), 8..15 = sum_l q2
            mv_s = ep.tile([P, C2T, B], bf16, tag="mv")

            def mm(out, lhsT, rhs, first, last):
                nc.tensor.matmul(out, lhsT, rhs, start=first, stop=last)

            def bc_recip(denom_ps):
                """[128,512] PSUM pre-broadcast softmax denominator ->
                [128,512] SBUF approx reciprocal (values in (~1e-28, 1e33):
                safely inside approx_fast's domain)."""
                bc = vp.tile([P, LC], f32, tag="bc", bufs=2)
                nc.vector.reciprocal_approx_fast(out=bc, in_=denom_ps)
                return bc

            def emit_ph1(xT_s, lc):
                ls = slice(lc * LC, (lc + 1) * LC)
                qT_s = tp.tile([P, DT, LC], f32r, tag="qt")
                for et in range(DT):
                    ps = pp.tile([P, LC], f32, tag="ps")
                    for dk in range(DT):
                        mm(ps, w1t_s[:, dk, et * P:(et + 1) * P],
                           xT_s[:, dk, ls], dk == 0, dk == DT - 1)
                    nc.scalar.copy(qT_s[:, et, :], ps)
                return qT_s

            def emit_ph2(xT_s, qT_s, b, lc):
                expT_s = tp.tile([P, LT, LC], bf16, tag="exp",
                                 name=f"exp1_{b}_{lc}")
                ps_d = pp.tile([P, LC], f32, tag="ps")
                for mt in range(LT):
                    ps = pp.tile([P, LC], f32, tag="ps")
                    for ek in range(DT):
                        mm(ps, xT_s[:, ek, mt * P:(mt + 1) * P],
                           qT_s[:, ek, :], ek == 0, ek == DT - 1)
                    nc.scalar.activation(expT_s[:, mt, :], ps, AF.Exp)
                    mm(ps_d, onesm_s, expT_s[:, mt, :], mt == 0, mt == LT - 1)
                return expT_s, ps_d

            def emit_ph3(x_s, expT_s, bc1, b, lc):
                mixT_s = tp.tile([P, DT, LC], f32r, tag="mix",
                                 name=f"mix_{b}_{lc}")
                for dt in range(DT):
                    ps = pp.tile([P, LC], f32, tag="ps")
                    for mk in range(LT):
                        mm(ps, x_s[:, mk, dt * P:(dt + 1) * P],
                           expT_s[:, mk, :], mk == 0, mk == LT - 1)
                    nc.vector.tensor_mul(mixT_s[:, dt, :], ps, bc1)
                return mixT_s

            def emit_ph4(mixT_s, qT_s, hTn_s, lc):
                """out1 -> tanh, written unnormalized into hTn[:, :, ls];
                ck-outer so the first matmuls only need wo1t chunk 0."""
                ls = slice(lc * LC, (lc + 1) * LC)
                for ot in range(DT):
                    ps = pp.tile([P, LC], f32, tag="ps")
                    for ck in range(D2T):
                        rhs = mixT_s[:, ck, :] if ck < DT else qT_s[:, ck - DT, :]
                        mm(ps, wo1t_s[:, ck, ot * P:(ot + 1) * P],
                           rhs, ck == 0, ck == D2T - 1)
                    nc.scalar.activation(hTn_s[:, ot, ls], ps, AF.Tanh)

            def emit_ph5(hTn_s, b, lc):
                """L2-normalize hTn[:, :, ls] in place (norm over the
                partition axis via ones-matmul)."""
                ls = slice(lc * LC, (lc + 1) * LC)
                hsq_s = tp.tile([P, DT, LC], bf16, tag="mix", name=f"hsq_{b}_{lc}")
                for dt in range(DT):
                    nc.scalar.activation(hsq_s[:, dt, :], hTn_s[:, dt, ls],
                                         AF.Square)
                ps_n = pp.tile([P, LC], f32, tag="ps")
                for dt in range(DT):
                    mm(ps_n, onesm_s, hsq_s[:, dt, :], dt == 0, dt == DT - 1)
                bcn = vp.tile([P, LC], f32, tag="bc", bufs=2, name=f"bcn_{b}_{lc}")
                bc2 = vp.tile([P, LC], f32, tag="bc", bufs=2, name=f"bc2_{b}_{lc}")
                nc.scalar.activation(bcn, ps_n, AF.Sqrt)
                nc.vector.tensor_scalar_max(bcn, bcn, 1e-12)
                nc.vector.reciprocal_approx_fast(out=bc2, in_=bcn)
                for dt in range(DT):
                    nc.vector.tensor_mul(hTn_s[:, dt, ls], hTn_s[:, dt, ls], bc2)

            def emit_ph7(hTn_s, xT_s, q2red_s, q2T_s, lc, et_lo, et_hi):
                ls = slice(lc * LC, (lc + 1) * LC)

                def c2T(k, fs):
                    return hTn_s[:, k, fs] if k < DT else xT_s[:, k - DT, fs]

                for et in range(et_lo, et_hi):
                    ps = pp.tile([P, LC], f32, tag="ps")
                    for dk in range(D2T):
                        mm(ps, w2t_s[:, dk, et * P:(et + 1) * P],
                           c2T(dk, ls), dk == 0, dk == D2T - 1)
                    nc.scalar.copy(q2T_s[:, et, :], ps)
                    # q2 column-sum partial, per et so it pipelines
                    # behind the copies instead of one monolithic reduce
                    with nc.allow_low_precision(reason="f32r rounding of sums"):
                        nc.vector.tensor_reduce(q2red_s[:, et, lc:lc + 1],
                                                q2T_s[:, et, :], axis=AXX,
                                                op=ALU.add)

            def emit_ph8_nat(hTn_s, xT_s, q2T_s, rrow_ps, b, lc):
                """Stage-2 attention in NATURAL orientation (query l on
                partitions): per l-tile, the softmax denominator is a free-
                axis DVE reduce and r accumulates via matmuls with the
                reciprocal vector as lhsT -- r = sum_lt u_lt^T @ exp2n_lt.
                No pre-broadcast denominator matmuls, no serial r block."""
                def c2T(k, fs):
                    return hTn_s[:, k, fs] if k < DT else xT_s[:, k - DT, fs]

                pend_u = [None]

                def flush_u():
                    if pend_u[0] is not None:
                        pu_b, pe2n, plt = pend_u[0]
                        pend_u[0] = None
                        for ms in range(NLC):
                            mm(rrow_ps[ms][0:1, :], pu_b, pe2n[:, ms, :],
                               plt == 0, plt == LT - 1)

                for li in range(LT // NLC):
                    lt = lc * (LT // NLC) + li
                    loff = li * P
                    e2n_s = tp.tile([P, NLC, LC], bf16, tag="e2n",
                                    name=f"e2n_{b}_{lt}", bufs=3)
                    dsum = vp.tile([P, 3], f32, tag="dsum", bufs=3,
                                   name=f"dsum_{b}_{lt}")
                    ps2 = [pp.tile([P, LC], f32, tag="ps",
                                   name=f"ps8_{b}_{lt}_{i}") for i in range(NLC)]
                    for ek in range(D2T):
                        for ms in range(NLC):
                            mm(ps2[ms], q2T_s[:, ek, loff:loff + P],
                               c2T(ek, slice(ms * LC, (ms + 1) * LC)),
                               ek == 0, ek == D2T - 1)
                    flush_u()   # previous lt's u-matmuls, now chain-covered
                    for ms in range(NLC):
                        nc.scalar.activation(e2n_s[:, ms, :], ps2[ms], AF.Exp)
                        nc.vector.tensor_reduce(dsum[:, ms:ms + 1],
                                                e2n_s[:, ms, :], axis=AXX,
                                                op=ALU.add)
                    nc.vector.tensor_reduce(dsum[:, 2:3], dsum[:, 0:2],
                                            axis=AXX, op=ALU.add)
                    u_f = vp.tile([P, 1], f32, tag="uf", bufs=3,
                                  name=f"uf_{b}_{lt}")
                    u_b = vp.tile([P, 1], bf16, tag="ub", bufs=3,
                                  name=f"ub_{b}_{lt}")
                    nc.vector.reciprocal_approx_fast(out=u_f, in_=dsum[:, 2:3])
                    with nc.allow_low_precision(reason="bf16 softmax scale"):
                        nc.vector.tensor_copy(u_b, u_f)
                    pend_u[0] = (u_b, e2n_s, lt)
                return flush_u

            def make_tail(b, hTn_s, xT_s, x_s, rrow_ps, q2red_s,
                          do_q2mv=True, fink=None):
                """Item tail, split in three so it can be emitted interleaved
                into the next item's stage-1 engine streams."""
                st = {}

                def tail_a():
                    if do_q2mv:
                        with nc.allow_low_precision(reason="f32r sums"):
                            nc.vector.tensor_reduce(mv_s[:, D2T:C2T, b:b + 1],
                                                    q2red_s, axis=AXX,
                                                    op=ALU.add)
                    rflat_s = vp.tile([1, L], bf16, tag="rflat", bufs=1,
                                      name=f"rflat_{b}")
                    nc.scalar.copy(rflat_s[0:1, 0:LC], rrow_ps[0][0:1, :])
                    nc.scalar.copy(rflat_s[0:1, LC:L], rrow_ps[1][0:1, :])
                    # r row -> column chunks: K=1 matmuls into disjoint
                    # columns of one psum bank
                    rc_ps = pp.tile([P, LT], f32, tag="ps", name=f"rc_{b}")
                    for mt in range(LT):
                        mm(rc_ps[:, mt:mt + 1],
                           rflat_s[0:1, mt * P:(mt + 1) * P],
                           onesr_s[0:1, 0:1], mt == 0, mt == LT - 1)
                    rsum_s = vp.tile([P, LT], bf16, tag="rsum", bufs=1,
                                     name=f"rsum_{b}")
                    with nc.allow_low_precision(reason="bf16 r"):
                        nc.vector.tensor_copy(rsum_s, rc_ps)
                    st["rflat"] = rflat_s
                    st["rsum"] = rsum_s

                def tail_b():
                    rbc_s = vp.tile([P, L], bf16, tag="rbc", bufs=1,
                                    name=f"rbc_{b}")
                    for j in range(NLC):
                        ps_b = pp.tile([P, LC], f32, tag="ps")
                        mm(ps_b, onesr_s, st["rflat"][0:1, j * LC:(j + 1) * LC],
                           True, True)
                        nc.scalar.copy(rbc_s[:, j * LC:(j + 1) * LC], ps_b)
                    st["rbc"] = rbc_s

                def tail_c():
                    rbc_s = st["rbc"]
                    rsum_s = st["rsum"]
                    with nc.allow_low_precision(reason="f32r rounding of sums"):
                        # x part: sum_m x[m,d] r[m] as tiny bf16 matmuls
                        for dt in range(DT):
                            ps_x = pp.tile([P, 1], f32, tag="ps",
                                           name=f"psx_{b}_{dt}")
                            for mk in range(LT):
                                mm(ps_x, x_s[:, mk, dt * P:(dt + 1) * P],
                                   rsum_s[:, mk:mk + 1], mk == 0, mk == LT - 1)
                            nc.vector.tensor_copy(mv_s[:, DT + dt, b:b + 1],
                                                  ps_x)
                            if fink:
                                fink(DT + dt, False)
                        # h part: transposed layout -> DVE weighted
                        # row-sums, split in halves for finer pipelining
                        hh = vp.tile([P, DT, 2], f32, tag="hh", bufs=1,
                                     name=f"hh_{b}")
                        for dt in range(DT):
                            for hf in range(2):
                                fs = slice(hf * LC, (hf + 1) * LC)
                                nc.vector.tensor_mul(hTn_s[:, dt, fs],
                                                     hTn_s[:, dt, fs], rbc_s[:, fs])
                                nc.vector.tensor_reduce(hh[:, dt, hf:hf + 1],
                                                        hTn_s[:, dt, fs],
                                                        axis=AXX, op=ALU.add)
                            nc.vector.tensor_reduce(mv_s[:, dt, b:b + 1],
                                                    hh[:, dt, :], axis=AXX,
                                                    op=ALU.add)
                            if fink:
                                fink(dt, dt == DT - 1)

                return tail_a, tail_b, tail_c

            pending = None
            nxt = None
            fin = {}
            nonlocal_state = {}
            for b in range(B):
                if nxt is None:
                    xT_s = xtp.tile([P, DT, L], f32r, tag="xT")
                    nc.sync.dma_start(out=w1t_s[:, 0:2, :],
                                      in_=w1t_ext[0:2 * P, :]
                                      .rearrange("(k p) e -> p k e", p=P))
                    nc.sync.dma_start(out=xT_s[:, 0:2, :],
                                      in_=xT_ext[b, 0:2 * P, :]
                                      .rearrange("(k p) l -> p k l", p=P))
                    nc.sync.dma_start(out=w1t_s[:, 2:DT, :],
                                      in_=w1t_ext[2 * P:DT * P, :]
                                      .rearrange("(k p) e -> p k e", p=P))
                    nc.sync.dma_start(out=xT_s[:, 2:DT, :],
                                      in_=xT_ext[b, 2 * P:DT * P, :]
                                      .rearrange("(k p) l -> p k l", p=P))
                    x_s = xp.tile([P, LT, D], bf16, tag="x")
                    nc.sync.dma_start(
                        out=x_s, in_=x_ext[b].rearrange("(k p) d -> p k d", p=P))
                    nc.sync.dma_start(
                        out=wo1t_s, in_=wo1t_ext.rearrange("(k p) e -> p k e", p=P))
                    nc.sync.dma_start(
                        out=w2t_s, in_=w2t_ext.rearrange("(k p) e -> p k e", p=P))
                    nc.sync.dma_start(
                        out=wo2t_s, in_=wo2t_ext.rearrange("(k p) e -> p k e", p=P))
                    qT0 = emit_ph1(xT_s, 0)
                else:
                    xT_s, x_s, qT0 = nxt
                    nxt = None
                hTn_s = hp.tile([P, DT, L], f32r, tag="hTn")
                q2red_s = vp.tile([P, D2T, NLC], f32r, tag="q2red", bufs=1,
                                  name=f"q2red_{b}")

                # ---- stage 1, lc0, with the previous item's tail
                # interleaved late enough that the PE stream has runway
                # before each tail matmul group
                exp0, psd0 = emit_ph2(xT_s, qT0, b, 0)
                bc1 = bc_recip(psd0)
                mix0 = emit_ph3(x_s, exp0, bc1, b, 0)
                if pending:
                    pending[0]()                    # r row extraction (PE+ACT)
                emit_ph4(mix0, qT0, hTn_s, 0)

                # ---- stage 1, lc1 (ph5 of lc0 slotted between PE phases)
                qT1 = emit_ph1(xT_s, 1)
                if pending:
                    pending[1]()                    # r broadcast (PE+ACT)
                exp1, psd1 = emit_ph2(xT_s, qT1, b, 1)
                emit_ph5(hTn_s, b, 0)
                bc1b = bc_recip(psd1)
                mix1 = emit_ph3(x_s, exp1, bc1b, b, 1)
                emit_ph4(mix1, qT1, hTn_s, 1)
                if pending:
                    pending[2]()                    # weighted row-sums (DVE)
                    pending = None

                # ---- stage 2 (ph5 of lc1 hidden behind ph7 of lc0;
                # ph7 of lc1 sliced into ph8(lc0)'s chain shadows; the next
                # item's DMA+ph1 (or the final q2-column matmuls) slice into
                # ph8(lc1)'s last chain shadow)
                rrow_ps = [pp.tile([P, LC], f32, tag="ps", name=f"rrow_{b}_{i}")
                           for i in range(NLC)]
                q2T0 = tp.tile([P, D2T, LC], f32r, tag="q2", name=f"q2_{b}_0")
                emit_ph7(hTn_s, xT_s, q2red_s, q2T0, 0, 0, D2T)
                emit_ph5(hTn_s, b, 1)
                pu0 = emit_ph8_nat(hTn_s, xT_s, q2T0, rrow_ps, b, 0)
                q2T1 = tp.tile([P, D2T, LC], f32r, tag="q2", name=f"q2_{b}_1")
                emit_ph7(hTn_s, xT_s, q2red_s, q2T1, 1, 0, 2)
                pu0()
                emit_ph7(hTn_s, xT_s, q2red_s, q2T1, 1, 2, D2T)

                if b < B - 1:
                    def head_next(bn=b + 1):
                        xTn = xtp.tile([P, DT, L], f32r, tag="xT")
                        nc.sync.dma_start(
                            out=xTn,
                            in_=xT_ext[bn].rearrange("(k p) l -> p k l", p=P))
                        xn = xp.tile([P, LT, D], bf16, tag="x")
                        nc.sync.dma_start(
                            out=xn,
                            in_=x_ext[bn].rearrange("(k p) d -> p k d", p=P))
                        nonlocal_state["nxt"] = (xTn, xn, emit_ph1(xTn, 0))
                    last_fill = head_next
                else:
                    def last_fill():
                        with nc.allow_low_precision(reason="f32r sums"):
                            nc.vector.tensor_reduce(mv_s[:, D2T:C2T, b:b + 1],
                                                    q2red_s, axis=AXX,
                                                    op=ALU.add)
                        emb_ps = pp.tile([P, LC], f32, tag="ps", name="emb_ps")
                        for i, ck in enumerate(range(D2T, C2T)):
                            mm(emb_ps[0:B, :], mv_s[:, ck, :], wo2t_s[:, ck, :],
                               i == 0, False)
                        fin["emb_ps"] = emb_ps
                pu1 = emit_ph8_nat(hTn_s, xT_s, q2T1, rrow_ps, b, 1)
                last_fill()
                pu1()
                if b < B - 1:
                    nxt = nonlocal_state.pop("nxt")

                def fink(ck, last, bb=b):
                    if bb == B - 1:
                        mm(fin["emb_ps"][0:B, :], mv_s[:, ck, :],
                           wo2t_s[:, ck, :], False, last)

                pending = make_tail(b, hTn_s, xT_s, x_s, rrow_ps, q2red_s,
                                    do_q2mv=(b < B - 1),
                                    fink=fink if b == B - 1 else None)

            # last item's tail (final Wo2 matmuls ride inside via fink)
            pending[0]()
            pending[1]()
            pending[2]()
            emb_ps = fin["emb_ps"]
            embf_s = vp.tile([B, D], f32, tag="bc", bufs=2, name="embf")
            nc.vector.tensor_copy(embf_s, emb_ps[0:B, :])
            nc.sync.dma_start(out=out_ext[:, :], in_=embf_s)

    _t1 = _time.time()
    nc.compile()
    print(f"[kernel] tile-trace+schedule {_t1 - _t0:.1f}s, "
          f"bacc compile {_time.time() - _t1:.1f}s", file=sys.stderr, flush=True)
    return nc


def get_nc():
    # the pipelined item tail reads xT(b) during item b+1, so the xT pool
    # MUST be double-buffered -- no xt_bufs=1 fallback (it deadlocks)
    if "nc" not in _CACHE:
        _CACHE["nc"] = _build_nc(xt_bufs=2)
    return _CACHE["nc"]


def make_in_maps(x, W1, Wo1, W2, Wo2):
    import ml_dtypes
    x = np.ascontiguousarray(np.asarray(x, dtype=np.float32))
    xT = np.ascontiguousarray(x.transpose(0, 2, 1))
    x_bf = np.ascontiguousarray(x.astype(ml_dtypes.bfloat16))
    w1t = np.ascontiguousarray(np.asarray(W1, np.float32).T)
    wo1t = np.ascontiguousarray(np.asarray(Wo1, np.float32).T)
    w2t = np.ascontiguousarray(np.asarray(W2, np.float32).T)
    # 1/L mean-scale folded into Wo2 (it only feeds the final matmuls)
    wo2t = np.ascontiguousarray((np.asarray(Wo2, np.float32).T / L).astype(ml_dtypes.bfloat16))
    onesm = np.ones((P, P), dtype=ml_dtypes.bfloat16)
    onesr = np.ones((1, P), dtype=ml_dtypes.bfloat16)
    return [
        {"x": x_bf[c * B:(c + 1) * B], "xT": xT[c * B:(c + 1) * B],
         "w1t": w1t, "wo1t": wo1t, "w2t": w2t, "wo2t": wo2t,
         "onesm": onesm, "onesr": onesr}
        for c in range(NCORES)
    ]


def run(x, W1, Wo1, W2, Wo2, trace=False, **kw):
    from concourse.bass_utils import run_bass_kernel_spmd
    nc = get_nc()
    in_maps = make_in_maps(x, W1, Wo1, W2, Wo2)
    res = run_bass_kernel_spmd(nc, in_maps, core_ids=list(range(NCORES)),
                               trace=trace, **kw)
    out = np.concatenate([res.results[c]["out"] for c in range(NCORES)], axis=0)
    return out.reshape(N_GLOBAL, D, 1, 1), res


def kernel(**inputs):
    out, _ = run(inputs["x"], inputs["W1"], inputs["Wo1"],
                 inputs["W2"], inputs["Wo2"])
    return out


# revision 31
# speedup vs baseline: 1.0188x; 1.0188x over previous
"""AttentionFuserV3 Trainium2 kernel: 8-core pure data parallel over batch.

Reference computation per batch item x_b [L=1024, D=512]:
  stage1: q = x W1^T; S = q x^T; A = softmax(S); mix = A x;
          h = tanh([mix, q] Wo1^T); h = h / max(||h||_2, eps)     (per row)
  stage2: c = [h, x]; q2 = c W2^T; S2 = q2 c^T; A2 = softmax(S2);
          mix2 = A2 c; o = [mix2, q2] Wo2^T; emb = mean_l(o)

Layout strategy ("T-space"): all big tensors are kept transposed in SBUF
(feature dim on partitions, sequence dim L on the free axis) so every
matmul contraction lands on the partition axis without on-device
transposes of the attention matrix.  Softmax runs without max-subtraction
(|scores| < ~70, exp stays in f32 range); softmax denominators are
accumulated pre-broadcast with a ones-matrix lhsT (M=128 costs the same
as M=1 on the PE) and inverted full-width with the fast approximate
reciprocal.

Stage-2 exploits linearity of the final mean:
  emb = mean_l(out2) = (1/L) [sum_l mix2 ; sum_l q2] Wo2^T
  sum_l mix2 = c2^T r   with   r[m] = sum_l exp2[m,l] / denom2[l]
so mix2 and out2 are never materialized per position; r is reduced on
DVE from the transposed exp2 tile, broadcast back with rank-1 matmuls,
and the Wo2 projection happens once for all batch items at the end.

The per-item tail (r extraction, broadcast, weighted row-sums) is
software-pipelined: its PE/DVE work is emitted interleaved into the NEXT
item's stage-1 stream so the PE never waits on the serial DVE chain.

Matmuls run in float32r (full PE speed at N=512); the attention
probabilities and the mix lhsT (x natural) are bf16.
"""

import sys

sys.path.insert(0, "/opt/trn_rl_repo")

import numpy as np

N_GLOBAL, L, D = 32, 1024, 512
NCORES = 8
B = N_GLOBAL // NCORES          # 4 batch items per core
P = 128
LC = 512                        # l-chunk (matmul moving free dim)
NLC = L // LC                   # 2
DT = D // P                     # 4
LT = L // P                     # 8
D2T = 2 * D // P                # 8
C2T = 4 * D // P                # 16

_CACHE = {}


def _build_nc(xt_bufs=2):
    import concourse.bass as bass  # noqa: F401
    import concourse.mybir as mybir
    import concourse.tile as tile
    from concourse import bacc

    f32 = mybir.dt.float32
    f32r = mybir.dt.float32r
    bf16 = mybir.dt.bfloat16
    AF = mybir.ActivationFunctionType
    ALU = mybir.AluOpType
    AXX = mybir.AxisListType.X

    nc = bacc.Bacc("TRN2", target_bir_lowering=False, debug=False,
                   num_devices=NCORES)

    x_ext = nc.declare_dram_parameter("x", [B, L, D], bf16, isOutput=False)
    xT_ext = nc.declare_dram_parameter("xT", [B, D, L], f32r, isOutput=False)
    w1t_ext = nc.declare_dram_parameter("w1t", [D, D], f32r, isOutput=False)
    wo1t_ext = nc.declare_dram_parameter("wo1t", [2 * D, D], f32r, isOutput=False)
    w2t_ext = nc.declare_dram_parameter("w2t", [2 * D, 2 * D], f32r, isOutput=False)
    wo2t_ext = nc.declare_dram_parameter("wo2t", [4 * D, D], bf16, isOutput=False)
    # Constants shipped from host: walrus's ISA check rejects memset/iota
    # writes into float32r tiles, but DMA from an f32r DRAM param is fine.
    onm_ext = nc.declare_dram_parameter("onesm", [P, P], bf16, isOutput=False)
    onr_ext = nc.declare_dram_parameter("onesr", [1, P], bf16, isOutput=False)
    out_ext = nc.declare_dram_parameter("out", [B, D], f32, isOutput=True)

    import time as _time
    _t0 = _time.time()
    with tile.TileContext(nc) as tc:
        with tc.tile_pool(name="wp", bufs=1) as wp, \
             tc.tile_pool(name="cp", bufs=1) as cp, \
             tc.tile_pool(name="xtp", bufs=xt_bufs) as xtp, \
             tc.tile_pool(name="xp", bufs=2) as xp, \
             tc.tile_pool(name="hp", bufs=2) as hp, \
             tc.tile_pool(name="tp", bufs=1) as tp, \
             tc.tile_pool(name="vp", bufs=2) as vp, \
             tc.tile_pool(name="ep", bufs=1) as ep, \
             tc.tile_pool(name="ps", bufs=8, space="PSUM") as pp:

            # ---- w1t gates ph1 of item 0: DMA it first (chunks of it
            # interleave with the xT chunks inside the b==0 iteration)
            w1t_s = wp.tile([P, DT, D], f32r, tag="w1t")
            onesm_s = cp.tile([P, P], bf16, tag="onesm")
            nc.sync.dma_start(out=onesm_s, in_=onm_ext[:, :])
            onesr_s = cp.tile([1, P], bf16, tag="onesr")
            nc.sync.dma_start(out=onesr_s, in_=onr_ext[:, :])

            wo1t_s = wp.tile([P, D2T, D], f32r, tag="wo1t")
            w2t_s = wp.tile([P, D2T, 2 * D], f32r, tag="w2t")
            wo2t_s = wp.tile([P, C2T, D], bf16, tag="wo2t")

            # meanvec columns for the deferred Wo2 projection:
            # c-chunks 0..3 = sum_l mix2 (h part), 4..7 (x part), 8..15 = sum_l q2
            mv_s = ep.tile([P, C2T, B], bf16, tag="mv")

            def mm(out, lhsT, rhs, first, last):
                nc.tensor.matmul(out, lhsT, rhs, start=first, stop=last)

            def bc_recip(denom_ps):
                """[128,512] PSUM pre-broadcast softmax denominator ->
                [128,512] SBUF approx reciprocal (values in (~1e-28, 1e33):
                safely inside approx_fast's domain)."""
                bc = vp.tile([P, LC], f32, tag="bc", bufs=2)
                nc.vector.reciprocal_approx_fast(out=bc, in_=denom_ps)
                return bc

            def emit_ph1(xT_s, lc):
                ls = slice(lc * LC, (lc + 1) * LC)
                qT_s = tp.tile([P, DT, LC], f32r, tag="qt")
                for et in range(DT):
                    ps = pp.tile([P, LC], f32, tag="ps")
                    for dk in range(DT):
                        mm(ps, w1t_s[:, dk, et * P:(et + 1) * P],
                           xT_s[:, dk, ls], dk == 0, dk == DT - 1)
                    nc.scalar.copy(qT_s[:, et, :], ps)
                return qT_s

            def emit_ph2(xT_s, qT_s, b, lc):
                expT_s = tp.tile([P, LT, LC], bf16, tag="exp",
                                 name=f"exp1_{b}_{lc}")
                ps_d = pp.tile([P, LC], f32, tag="ps")
                for mt in range(LT):
                    ps = pp.tile([P, LC], f32, tag="ps")
                    for ek in range(DT):
                        mm(ps, xT_s[:, ek, mt * P:(mt + 1) * P],
                           qT_s[:, ek, :], ek == 0, ek == DT - 1)
                    nc.scalar.activation(expT_s[:, mt, :], ps, AF.Exp)
                    mm(ps_d, onesm_s, expT_s[:, mt, :], mt == 0, mt == LT - 1)
                return expT_s, ps_d

            def emit_ph3(x_s, expT_s, bc1, b, lc):
                mixT_s = tp.tile([P, DT, LC], f32r, tag="mix",
                                 name=f"mix_{b}_{lc}")
                for dt in range(DT):
                    ps = pp.tile([P, LC], f32, tag="ps")
                    for mk in range(LT):
                        mm(ps, x_s[:, mk, dt * P:(dt + 1) * P],
                           expT_s[:, mk, :], mk == 0, mk == LT - 1)
                    nc.vector.tensor_mul(mixT_s[:, dt, :], ps, bc1)
                return mixT_s

            def emit_ph4(mixT_s, qT_s, hTn_s, lc):
                """out1 -> tanh, written unnormalized into hTn[:, :, ls];
                ck-outer so the first matmuls only need wo1t chunk 0."""
                ls = slice(lc * LC, (lc + 1) * LC)
                for ot in range(DT):
                    ps = pp.tile([P, LC], f32, tag="ps")
                    for ck in range(D2T):
                        rhs = mixT_s[:, ck, :] if ck < DT else qT_s[:, ck - DT, :]
                        mm(ps, wo1t_s[:, ck, ot * P:(ot + 1) * P],
                           rhs, ck == 0, ck == D2T - 1)
                    nc.scalar.activation(hTn_s[:, ot, ls], ps, AF.Tanh)

            def emit_ph5(hTn_s, b, lc):
                """L2-normalize hTn[:, :, ls] in place (norm over the
                partition axis via ones-matmul)."""
                ls = slice(lc * LC, (lc + 1) * LC)
                hsq_s = tp.tile([P, DT, LC], bf16, tag="mix", name=f"hsq_{b}_{lc}")
                for dt in range(DT):
                    nc.vector.tensor_mul(hsq_s[:, dt, :], hTn_s[:, dt, ls],
                                         hTn_s[:, dt, ls])
                ps_n = pp.tile([P, LC], f32, tag="ps")
                for dt in range(DT):
                    mm(ps_n, onesm_s, hsq_s[:, dt, :], dt == 0, dt == DT - 1)
                bcn = vp.tile([P, LC], f32, tag="bc", bufs=2, name=f"bcn_{b}_{lc}")
                bc2 = vp.tile([P, LC], f32, tag="bc", bufs=2, name=f"bc2_{b}_{lc}")
                nc.scalar.activation(bcn, ps_n, AF.Sqrt)
                nc.vector.tensor_scalar_max(bcn, bcn, 1e-12)
                nc.vector.reciprocal_approx_fast(out=bc2, in_=bcn)
                for dt in range(DT):
                    nc.vector.tensor_mul(hTn_s[:, dt, ls], hTn_s[:, dt, ls], bc2)

            def emit_ph7(hTn_s, xT_s, q2red_s, q2T_s, lc, et_lo, et_hi):
                ls = slice(lc * LC, (lc + 1) * LC)

                def c2T(k, fs):
                    return hTn_s[:, k, fs] if k < DT else xT_s[:, k - DT, fs]

                for et in range(et_lo, et_hi):
                    ps = pp.tile([P, LC], f32, tag="ps")
                    for dk in range(D2T):
                        mm(ps, w2t_s[:, dk, et * P:(et + 1) * P],
                           c2T(dk, ls), dk == 0, dk == D2T - 1)
                    nc.scalar.copy(q2T_s[:, et, :], ps)
                    # q2 column-sum partial, per et so it pipelines
                    # behind the copies instead of one monolithic reduce
                    with nc.allow_low_precision(reason="f32r rounding of sums"):
                        nc.vector.tensor_reduce(q2red_s[:, et, lc:lc + 1],
                                                q2T_s[:, et, :], axis=AXX,
                                                op=ALU.add)

            def emit_ph8_nat(hTn_s, xT_s, q2T_s, rrow_ps, b, lc):
                """Stage-2 attention in NATURAL orientation (query l on
                partitions): per l-tile, the softmax denominator is a free-
                axis DVE reduce and r accumulates via matmuls with the
                reciprocal vector as lhsT -- r = sum_lt u_lt^T @ exp2n_lt.
                No pre-broadcast denominator matmuls, no serial r block."""
                def c2T(k, fs):
                    return hTn_s[:, k, fs] if k < DT else xT_s[:, k - DT, fs]

                pend_u = [None]

                def flush_u():
                    if pend_u[0] is not None:
                        pu_b, pe2n, plt = pend_u[0]
                        pend_u[0] = None
                        for ms in range(NLC):
                            mm(rrow_ps[ms][0:1, :], pu_b, pe2n[:, ms, :],
                               plt == 0, plt == LT - 1)

                for li in range(LT // NLC):
                    lt = lc * (LT // NLC) + li
                    loff = li * P
                    e2n_s = tp.tile([P, NLC, LC], bf16, tag="e2n",
                                    name=f"e2n_{b}_{lt}", bufs=3)
                    dsum = vp.tile([P, 3], f32, tag="dsum", bufs=3,
                                   name=f"dsum_{b}_{lt}")
                    ps2 = [pp.tile([P, LC], f32, tag="ps",
                                   name=f"ps8_{b}_{lt}_{i}") for i in range(NLC)]
                    for ek in range(D2T):
                        for ms in range(NLC):
                            mm(ps2[ms], q2T_s[:, ek, loff:loff + P],
                               c2T(ek, slice(ms * LC, (ms + 1) * LC)),
                               ek == 0, ek == D2T - 1)
                    flush_u()   # previous lt's u-matmuls, now chain-covered
                    for ms in range(NLC):
                        nc.scalar.activation(e2n_s[:, ms, :], ps2[ms], AF.Exp)
                        nc.vector.tensor_reduce(dsum[:, ms:ms + 1],
                                                e2n_s[:, ms, :], axis=AXX,
                                                op=ALU.add)
                    nc.vector.tensor_reduce(dsum[:, 2:3], dsum[:, 0:2],
                                            axis=AXX, op=ALU.add)
                    u_f = vp.tile([P, 1], f32, tag="uf", bufs=3,
                                  name=f"uf_{b}_{lt}")
                    u_b = vp.tile([P, 1], bf16, tag="ub", bufs=3,
                                  name=f"ub_{b}_{lt}")
                    nc.vector.reciprocal_approx_fast(out=u_f, in_=dsum[:, 2:3])
                    with nc.allow_low_precision(reason="bf16 softmax scale"):
                        nc.vector.tensor_copy(u_b, u_f)
                    pend_u[0] = (u_b, e2n_s, lt)
                return flush_u

            def make_tail(b, hTn_s, xT_s, x_s, rrow_ps, q2red_s,
                          do_q2mv=True, fink=None):
                """Item tail, split in three so it can be emitted interleaved
                into the next item's stage-1 engine streams."""
                st = {}

                def tail_a():
                    if do_q2mv:
                        with nc.allow_low_precision(reason="f32r sums"):
                            nc.vector.tensor_reduce(mv_s[:, D2T:C2T, b:b + 1],
                                                    q2red_s, axis=AXX,
                                                    op=ALU.add)
                    rflat_s = vp.tile([1, L], bf16, tag="rflat", bufs=1,
                                      name=f"rflat_{b}")
                    nc.scalar.copy(rflat_s[0:1, 0:LC], rrow_ps[0][0:1, :])
                    nc.scalar.copy(rflat_s[0:1, LC:L], rrow_ps[1][0:1, :])
                    # r row -> column chunks: K=1 matmuls into disjoint
                    # columns of one psum bank
                    rc_ps = pp.tile([P, LT], f32, tag="ps", name=f"rc_{b}")
                    for mt in range(LT):
                        mm(rc_ps[:, mt:mt + 1],
                           rflat_s[0:1, mt * P:(mt + 1) * P],
                           onesr_s[0:1, 0:1], mt == 0, mt == LT - 1)
                    rsum_s = vp.tile([P, LT], bf16, tag="rsum", bufs=1,
                                     name=f"rsum_{b}")
                    with nc.allow_low_precision(reason="bf16 r"):
                        nc.vector.tensor_copy(rsum_s, rc_ps)
                    st["rflat"] = rflat_s
                    st["rsum"] = rsum_s

                def tail_b():
                    rbc_s = vp.tile([P, L], bf16, tag="rbc", bufs=1,
                                    name=f"rbc_{b}")
                    for j in range(NLC):
                        ps_b = pp.tile([P, LC], f32, tag="ps")
                        mm(ps_b, onesr_s, st["rflat"][0:1, j * LC:(j + 1) * LC],
                           True, True)
                        nc.scalar.copy(rbc_s[:, j * LC:(j + 1) * LC], ps_b)
                    st["rbc"] = rbc_s

                def tail_c():
                    rbc_s = st["rbc"]
                    rsum_s = st["rsum"]
                    with nc.allow_low_precision(reason="f32r rounding of sums"):
                        # x part: sum_m x[m,d] r[m] as tiny bf16 matmuls
                        for dt in range(DT):
                            ps_x = pp.tile([P, 1], f32, tag="ps",
                                           name=f"psx_{b}_{dt}")
                            for mk in range(LT):
                                mm(ps_x, x_s[:, mk, dt * P:(dt + 1) * P],
                                   rsum_s[:, mk:mk + 1], mk == 0, mk == LT - 1)
                            nc.vector.tensor_copy(mv_s[:, DT + dt, b:b + 1],
                                                  ps_x)
                            if fink:
                                fink(DT + dt, False)
                        # h part: transposed layout -> DVE weighted
                        # row-sums, split in halves for finer pipelining
                        hh = vp.tile([P, DT, 2], f32, tag="hh", bufs=1,
                                     name=f"hh_{b}")
                        for dt in range(DT):
                            for hf in range(2):
                                fs = slice(hf * LC, (hf + 1) * LC)
                                nc.vector.tensor_mul(hTn_s[:, dt, fs],
                                                     hTn_s[:, dt, fs], rbc_s[:, fs])
                                nc.vector.tensor_reduce(hh[:, dt, hf:hf + 1],
                                                        hTn_s[:, dt, fs],
                                                        axis=AXX, op=ALU.add)
                            nc.vector.tensor_reduce(mv_s[:, dt, b:b + 1],
                                                    hh[:, dt, :], axis=AXX,
                                                    op=ALU.add)
                            if fink:
                                fink(dt, dt == DT - 1)

                return tail_a, tail_b, tail_c

            pending = None
            nxt = None
            fin = {}
            nonlocal_state = {}
            for b in range(B):
                if nxt is None:
                    xT_s = xtp.tile([P, DT, L], f32r, tag="xT")
                    nc.sync.dma_start(out=w1t_s[:, 0:2, :],
                                      in_=w1t_ext[0:2 * P, :]
                                      .rearrange("(k p) e -> p k e", p=P))
                    nc.sync.dma_start(out=xT_s[:, 0:2, :],
                                      in_=xT_ext[b, 0:2 * P, :]
                                      .rearrange("(k p) l -> p k l", p=P))
                    nc.sync.dma_start(out=w1t_s[:, 2:DT, :],
                                      in_=w1t_ext[2 * P:DT * P, :]
                                      .rearrange("(k p) e -> p k e", p=P))
                    nc.sync.dma_start(out=xT_s[:, 2:DT, :],
                                      in_=xT_ext[b, 2 * P:DT * P, :]
                                      .rearrange("(k p) l -> p k l", p=P))
                    x_s = xp.tile([P, LT, D], bf16, tag="x")
                    nc.sync.dma_start(
                        out=x_s, in_=x_ext[b].rearrange("(k p) d -> p k d", p=P))
                    nc.sync.dma_start(
                        out=wo1t_s, in_=wo1t_ext.rearrange("(k p) e -> p k e", p=P))
                    nc.sync.dma_start(
                        out=w2t_s, in_=w2t_ext.rearrange("(k p) e -> p k e", p=P))
                    nc.sync.dma_start(
                        out=wo2t_s, in_=wo2t_ext.rearrange("(k p) e -> p k e", p=P))
                    qT0 = emit_ph1(xT_s, 0)
                else:
                    xT_s, x_s, qT0 = nxt
                    nxt = None
                hTn_s = hp.tile([P, DT, L], f32r, tag="hTn")
                q2red_s = vp.tile([P, D2T, NLC], f32r, tag="q2red", bufs=1,
                                  name=f"q2red_{b}")

                # ---- stage 1, lc0, with the previous item's tail
                # interleaved late enough that the PE stream has runway
                # before each tail matmul group
                exp0, psd0 = emit_ph2(xT_s, qT0, b, 0)
                bc1 = bc_recip(psd0)
                mix0 = emit_ph3(x_s, exp0, bc1, b, 0)
                if pending:
                    pending[0]()                    # r row extraction (PE+ACT)
                emit_ph4(mix0, qT0, hTn_s, 0)

                # ---- stage 1, lc1 (ph5 of lc0 slotted between PE phases)
                qT1 = emit_ph1(xT_s, 1)
                if pending:
                    pending[1]()                    # r broadcast (PE+ACT)
                exp1, psd1 = emit_ph2(xT_s, qT1, b, 1)
                emit_ph5(hTn_s, b, 0)
                bc1b = bc_recip(psd1)
                mix1 = emit_ph3(x_s, exp1, bc1b, b, 1)
                emit_ph4(mix1, qT1, hTn_s, 1)
                if pending:
                    pending[2]()                    # weighted row-sums (DVE)
                    pending = None

                # ---- stage 2 (ph5 of lc1 hidden behind ph7 of lc0;
                # ph7 of lc1 sliced into ph8(lc0)'s chain shadows; the next
                # item's DMA+ph1 (or the final q2-column matmuls) slice into
                # ph8(lc1)'s last chain shadow)
                rrow_ps = [pp.tile([P, LC], f32, tag="ps", name=f"rrow_{b}_{i}")
                           for i in range(NLC)]
                q2T0 = tp.tile([P, D2T, LC], f32r, tag="q2", name=f"q2_{b}_0")
                emit_ph7(hTn_s, xT_s, q2red_s, q2T0, 0, 0, 4)
                emit_ph5(hTn_s, b, 1)
                emit_ph7(hTn_s, xT_s, q2red_s, q2T0, 0, 4, D2T)
                pu0 = emit_ph8_nat(hTn_s, xT_s, q2T0, rrow_ps, b, 0)
                q2T1 = tp.tile([P, D2T, LC], f32r, tag="q2", name=f"q2_{b}_1")
                emit_ph7(hTn_s, xT_s, q2red_s, q2T1, 1, 0, 2)
                pu0()
                emit_ph7(hTn_s, xT_s, q2red_s, q2T1, 1, 2, D2T)

                if b < B - 1:
                    def head_next(bn=b + 1):
                        xTn = xtp.tile([P, DT, L], f32r, tag="xT")
                        nc.sync.dma_start(
                            out=xTn,
                            in_=xT_ext[bn].rearrange("(k p) l -> p k l", p=P))
                        xn = xp.tile([P, LT, D], bf16, tag="x")
                        nc.sync.dma_start(
                            out=xn,
                            in_=x_ext[bn].rearrange("(k p) d -> p k d", p=P))
                        nonlocal_state["nxt"] = (xTn, xn, emit_ph1(xTn, 0))
                    last_fill = head_next
                else:
                    def last_fill():
                        with nc.allow_low_precision(reason="f32r sums"):
                            nc.vector.tensor_reduce(mv_s[:, D2T:C2T, b:b + 1],
                                                    q2red_s, axis=AXX,
                                                    op=ALU.add)
                        emb_ps = pp.tile([P, LC], f32, tag="ps", name="emb_ps")
                        for i, ck in enumerate(range(D2T, C2T)):
                            mm(emb_ps[0:B, :], mv_s[:, ck, :], wo2t_s[:, ck, :],
                               i == 0, False)
                        fin["emb_ps"] = emb_ps
                pu1 = emit_ph8_nat(hTn_s, xT_s, q2T1, rrow_ps, b, 1)
                last_fill()
                pu1()
                if b < B - 1:
                    nxt = nonlocal_state.pop("nxt")

                def fink(ck, last, bb=b):
                    if bb == B - 1:
                        mm(fin["emb_ps"][0:B, :], mv_s[:, ck, :],
                           wo2t_s[:, ck, :], False, last)

                pending = make_tail(b, hTn_s, xT_s, x_s, rrow_ps, q2red_s,
                                    do_q2mv=(b < B - 1),
                                    fink=fink if b == B - 1 else None)

            # last item's tail (final Wo2 matmuls ride inside via fink)
            pending[0]()
            pending[1]()
            pending[2]()
            emb_ps = fin["emb_ps"]
            embf_s = vp.tile([B, D], f32, tag="bc", bufs=2, name="embf")
            nc.vector.tensor_copy(embf_s, emb_ps[0:B, :])
            nc.sync.dma_start(out=out_ext[:, :], in_=embf_s)

    _t1 = _time.time()
    nc.compile()
    print(f"[kernel] tile-trace+schedule {_t1 - _t0:.1f}s, "
          f"bacc compile {_time.time() - _t1:.1f}s", file=sys.stderr, flush=True)
    return nc


def get_nc():
    # the pipelined item tail reads xT(b) during item b+1, so the xT pool
    # MUST be double-buffered -- no xt_bufs=1 fallback (it deadlocks)
    if "nc" not in _CACHE:
        _CACHE["nc"] = _build_nc(xt_bufs=2)
    return _CACHE["nc"]


def make_in_maps(x, W1, Wo1, W2, Wo2):
    import ml_dtypes
    x = np.ascontiguousarray(np.asarray(x, dtype=np.float32))
    xT = np.ascontiguousarray(x.transpose(0, 2, 1))
    x_bf = np.ascontiguousarray(x.astype(ml_dtypes.bfloat16))
    w1t = np.ascontiguousarray(np.asarray(W1, np.float32).T)
    wo1t = np.ascontiguousarray(np.asarray(Wo1, np.float32).T)
    w2t = np.ascontiguousarray(np.asarray(W2, np.float32).T)
    # 1/L mean-scale folded into Wo2 (it only feeds the final matmuls)
    wo2t = np.ascontiguousarray((np.asarray(Wo2, np.float32).T / L).astype(ml_dtypes.bfloat16))
    onesm = np.ones((P, P), dtype=ml_dtypes.bfloat16)
    onesr = np.ones((1, P), dtype=ml_dtypes.bfloat16)
    return [
        {"x": x_bf[c * B:(c + 1) * B], "xT": xT[c * B:(c + 1) * B],
         "w1t": w1t, "wo1t": wo1t, "w2t": w2t, "wo2t": wo2t,
         "onesm": onesm, "onesr": onesr}
        for c in range(NCORES)
    ]


def run(x, W1, Wo1, W2, Wo2, trace=False, **kw):
    from concourse.bass_utils import run_bass_kernel_spmd
    nc = get_nc()
    in_maps = make_in_maps(x, W1, Wo1, W2, Wo2)
    res = run_bass_kernel_spmd(nc, in_maps, core_ids=list(range(NCORES)),
                               trace=trace, **kw)
    out = np.concatenate([res.results[c]["out"] for c in range(NCORES)], axis=0)
    return out.reshape(N_GLOBAL, D, 1, 1), res


def kernel(**inputs):
    out, _ = run(inputs["x"], inputs["W1"], inputs["Wo1"],
                 inputs["W2"], inputs["Wo2"])
    return out


# revision 32
# speedup vs baseline: 1.0202x; 1.0013x over previous
"""AttentionFuserV3 Trainium2 kernel: 8-core pure data parallel over batch.

Reference computation per batch item x_b [L=1024, D=512]:
  stage1: q = x W1^T; S = q x^T; A = softmax(S); mix = A x;
          h = tanh([mix, q] Wo1^T); h = h / max(||h||_2, eps)     (per row)
  stage2: c = [h, x]; q2 = c W2^T; S2 = q2 c^T; A2 = softmax(S2);
          mix2 = A2 c; o = [mix2, q2] Wo2^T; emb = mean_l(o)

Layout strategy ("T-space"): all big tensors are kept transposed in SBUF
(feature dim on partitions, sequence dim L on the free axis) so every
matmul contraction lands on the partition axis without on-device
transposes of the attention matrix.  Softmax runs without max-subtraction
(|scores| < ~70, exp stays in f32 range); softmax denominators are
accumulated pre-broadcast with a ones-matrix lhsT (M=128 costs the same
as M=1 on the PE) and inverted full-width with the fast approximate
reciprocal.

Stage-2 exploits linearity of the final mean:
  emb = mean_l(out2) = (1/L) [sum_l mix2 ; sum_l q2] Wo2^T
  sum_l mix2 = c2^T r   with   r[m] = sum_l exp2[m,l] / denom2[l]
so mix2 and out2 are never materialized per position; r is reduced on
DVE from the transposed exp2 tile, broadcast back with rank-1 matmuls,
and the Wo2 projection happens once for all batch items at the end.

The per-item tail (r extraction, broadcast, weighted row-sums) is
software-pipelined: its PE/DVE work is emitted interleaved into the NEXT
item's stage-1 stream so the PE never waits on the serial DVE chain.

Matmuls run in float32r (full PE speed at N=512); the attention
probabilities and the mix lhsT (x natural) are bf16.
"""

import sys

sys.path.insert(0, "/opt/trn_rl_repo")

import numpy as np

N_GLOBAL, L, D = 32, 1024, 512
NCORES = 8
B = N_GLOBAL // NCORES          # 4 batch items per core
P = 128
LC = 512                        # l-chunk (matmul moving free dim)
NLC = L // LC                   # 2
DT = D // P                     # 4
LT = L // P                     # 8
D2T = 2 * D // P                # 8
C2T = 4 * D // P                # 16

_CACHE = {}


def _build_nc(xt_bufs=2):
    import concourse.bass as bass  # noqa: F401
    import concourse.mybir as mybir
    import concourse.tile as tile
    from concourse import bacc

    f32 = mybir.dt.float32
    f32r = mybir.dt.float32r
    bf16 = mybir.dt.bfloat16
    AF = mybir.ActivationFunctionType
    ALU = mybir.AluOpType
    AXX = mybir.AxisListType.X

    nc = bacc.Bacc("TRN2", target_bir_lowering=False, debug=False,
                   num_devices=NCORES)

    x_ext = nc.declare_dram_parameter("x", [B, L, D], bf16, isOutput=False)
    xT_ext = nc.declare_dram_parameter("xT", [B, D, L], f32r, isOutput=False)
    w1t_ext = nc.declare_dram_parameter("w1t", [D, D], f32r, isOutput=False)
    wo1t_ext = nc.declare_dram_parameter("wo1t", [2 * D, D], f32r, isOutput=False)
    w2t_ext = nc.declare_dram_parameter("w2t", [2 * D, 2 * D], f32r, isOutput=False)
    wo2t_ext = nc.declare_dram_parameter("wo2t", [4 * D, D], bf16, isOutput=False)
    # Constants shipped from host: walrus's ISA check rejects memset/iota
    # writes into float32r tiles, but DMA from an f32r DRAM param is fine.
    onm_ext = nc.declare_dram_parameter("onesm", [P, P], bf16, isOutput=False)
    onr_ext = nc.declare_dram_parameter("onesr", [1, P], bf16, isOutput=False)
    out_ext = nc.declare_dram_parameter("out", [B, D], f32, isOutput=True)

    import time as _time
    _t0 = _time.time()
    with tile.TileContext(nc) as tc:
        with tc.tile_pool(name="wp", bufs=1) as wp, \
             tc.tile_pool(name="cp", bufs=1) as cp, \
             tc.tile_pool(name="xtp", bufs=xt_bufs) as xtp, \
             tc.tile_pool(name="xp", bufs=2) as xp, \
             tc.tile_pool(name="hp", bufs=2) as hp, \
             tc.tile_pool(name="tp", bufs=1) as tp, \
             tc.tile_pool(name="vp", bufs=2) as vp, \
             tc.tile_pool(name="ep", bufs=1) as ep, \
             tc.tile_pool(name="ps", bufs=8, space="PSUM") as pp:

            # ---- w1t gates ph1 of item 0: DMA it first (chunks of it
            # interleave with the xT chunks inside the b==0 iteration)
            w1t_s = wp.tile([P, DT, D], f32r, tag="w1t")
            onesm_s = cp.tile([P, P], bf16, tag="onesm")
            nc.sync.dma_start(out=onesm_s, in_=onm_ext[:, :])
            onesr_s = cp.tile([1, P], bf16, tag="onesr")
            nc.sync.dma_start(out=onesr_s, in_=onr_ext[:, :])

            wo1t_s = wp.tile([P, D2T, D], f32r, tag="wo1t")
            w2t_s = wp.tile([P, D2T, 2 * D], f32r, tag="w2t")
            wo2t_s = wp.tile([P, C2T, D], bf16, tag="wo2t")

            # meanvec columns for the deferred Wo2 projection:
            # c-chunks 0..3 = sum_l mix2 (h part), 4..7 (x part), 8..15 = sum_l q2
            mv_s = ep.tile([P, C2T, B], bf16, tag="mv")

            def mm(out, lhsT, rhs, first, last):
                nc.tensor.matmul(out, lhsT, rhs, start=first, stop=last)

            def bc_recip(denom_ps):
                """[128,512] PSUM pre-broadcast softmax denominator ->
                [128,512] SBUF approx reciprocal (values in (~1e-28, 1e33):
                safely inside approx_fast's domain)."""
                bc = vp.tile([P, LC], f32, tag="bc", bufs=2)
                nc.vector.reciprocal_approx_fast(out=bc, in_=denom_ps)
                return bc

            def emit_ph1(xT_s, lc):
                ls = slice(lc * LC, (lc + 1) * LC)
                qT_s = tp.tile([P, DT, LC], f32r, tag="qt")
                for et in range(DT):
                    ps = pp.tile([P, LC], f32, tag="ps")
                    for dk in range(DT):
                        mm(ps, w1t_s[:, dk, et * P:(et + 1) * P],
                           xT_s[:, dk, ls], dk == 0, dk == DT - 1)
                    nc.scalar.copy(qT_s[:, et, :], ps)
                return qT_s

            def emit_ph2(xT_s, qT_s, b, lc):
                expT_s = tp.tile([P, LT, LC], bf16, tag="exp",
                                 name=f"exp1_{b}_{lc}")
                ps_d = pp.tile([P, LC], f32, tag="ps")
                for mt in range(LT):
                    ps = pp.tile([P, LC], f32, tag="ps")
                    for ek in range(DT):
                        mm(ps, xT_s[:, ek, mt * P:(mt + 1) * P],
                           qT_s[:, ek, :], ek == 0, ek == DT - 1)
                    nc.scalar.activation(expT_s[:, mt, :], ps, AF.Exp)
                    mm(ps_d, onesm_s, expT_s[:, mt, :], mt == 0, mt == LT - 1)
                return expT_s, ps_d

            def emit_ph3(x_s, expT_s, bc1, b, lc):
                mixT_s = tp.tile([P, DT, LC], f32r, tag="mix",
                                 name=f"mix_{b}_{lc}")
                for dt in range(DT):
                    ps = pp.tile([P, LC], f32, tag="ps")
                    for mk in range(LT):
                        mm(ps, x_s[:, mk, dt * P:(dt + 1) * P],
                           expT_s[:, mk, :], mk == 0, mk == LT - 1)
                    nc.vector.tensor_mul(mixT_s[:, dt, :], ps, bc1)
                return mixT_s

            def emit_ph4(mixT_s, qT_s, hTn_s, lc):
                """out1 -> tanh, written unnormalized into hTn[:, :, ls];
                ck-outer so the first matmuls only need wo1t chunk 0."""
                ls = slice(lc * LC, (lc + 1) * LC)
                for ot in range(DT):
                    ps = pp.tile([P, LC], f32, tag="ps")
                    for ck in range(D2T):
                        rhs = mixT_s[:, ck, :] if ck < DT else qT_s[:, ck - DT, :]
                        mm(ps, wo1t_s[:, ck, ot * P:(ot + 1) * P],
                           rhs, ck == 0, ck == D2T - 1)
                    nc.scalar.activation(hTn_s[:, ot, ls], ps, AF.Tanh)

            def emit_ph5(hTn_s, b, lc):
                """L2-normalize hTn[:, :, ls] in place (norm over the
                partition axis via ones-matmul)."""
                ls = slice(lc * LC, (lc + 1) * LC)
                hsq_s = tp.tile([P, DT, LC], bf16, tag="mix", name=f"hsq_{b}_{lc}")
                for dt in range(DT):
                    nc.vector.tensor_mul(hsq_s[:, dt, :], hTn_s[:, dt, ls],
                                         hTn_s[:, dt, ls])
                ps_n = pp.tile([P, LC], f32, tag="ps")
                for dt in range(DT):
                    mm(ps_n, onesm_s, hsq_s[:, dt, :], dt == 0, dt == DT - 1)
                bcn = vp.tile([P, LC], f32, tag="bc", bufs=2, name=f"bcn_{b}_{lc}")
                bc2 = vp.tile([P, LC], f32, tag="bc", bufs=2, name=f"bc2_{b}_{lc}")
                nc.scalar.activation(bcn, ps_n, AF.Sqrt)
                nc.vector.tensor_scalar_max(bcn, bcn, 1e-12)
                nc.vector.reciprocal_approx_fast(out=bc2, in_=bcn)
                for dt in range(DT):
                    nc.vector.tensor_mul(hTn_s[:, dt, ls], hTn_s[:, dt, ls], bc2)

            def emit_ph7(hTn_s, xT_s, q2red_s, q2T_s, lc, et_lo, et_hi):
                ls = slice(lc * LC, (lc + 1) * LC)

                def c2T(k, fs):
                    return hTn_s[:, k, fs] if k < DT else xT_s[:, k - DT, fs]

                for et in range(et_lo, et_hi):
                    ps = pp.tile([P, LC], f32, tag="ps")
                    for dk in range(D2T):
                        mm(ps, w2t_s[:, dk, et * P:(et + 1) * P],
                           c2T(dk, ls), dk == 0, dk == D2T - 1)
                    nc.scalar.copy(q2T_s[:, et, :], ps)
                    # q2 column-sum partial, per et so it pipelines
                    # behind the copies instead of one monolithic reduce
                    with nc.allow_low_precision(reason="f32r rounding of sums"):
                        nc.vector.tensor_reduce(q2red_s[:, et, lc:lc + 1],
                                                q2T_s[:, et, :], axis=AXX,
                                                op=ALU.add)

            def emit_ph8_nat(hTn_s, xT_s, q2T_s, rrow_ps, b, lc):
                """Stage-2 attention in NATURAL orientation (query l on
                partitions): per l-tile, the softmax denominator is a free-
                axis DVE reduce and r accumulates via matmuls with the
                reciprocal vector as lhsT -- r = sum_lt u_lt^T @ exp2n_lt.
                No pre-broadcast denominator matmuls, no serial r block."""
                def c2T(k, fs):
                    return hTn_s[:, k, fs] if k < DT else xT_s[:, k - DT, fs]

                pend_u = [None]

                def flush_u():
                    if pend_u[0] is not None:
                        pu_b, pe2n, plt = pend_u[0]
                        pend_u[0] = None
                        for ms in range(NLC):
                            mm(rrow_ps[ms][0:1, :], pu_b, pe2n[:, ms, :],
                               plt == 0, plt == LT - 1)

                for li in range(LT // NLC):
                    lt = lc * (LT // NLC) + li
                    loff = li * P
                    e2n_s = tp.tile([P, NLC, LC], bf16, tag="e2n",
                                    name=f"e2n_{b}_{lt}", bufs=3)
                    dsum = vp.tile([P, 3], f32, tag="dsum", bufs=3,
                                   name=f"dsum_{b}_{lt}")
                    ps2 = [pp.tile([P, LC], f32, tag="ps",
                                   name=f"ps8_{b}_{lt}_{i}") for i in range(NLC)]
                    for ek in range(D2T):
                        for ms in range(NLC):
                            mm(ps2[ms], q2T_s[:, ek, loff:loff + P],
                               c2T(ek, slice(ms * LC, (ms + 1) * LC)),
                               ek == 0, ek == D2T - 1)
                    flush_u()   # previous lt's u-matmuls, now chain-covered
                    for ms in range(NLC):
                        nc.scalar.activation(e2n_s[:, ms, :], ps2[ms], AF.Exp)
                        nc.vector.tensor_reduce(dsum[:, ms:ms + 1],
                                                e2n_s[:, ms, :], axis=AXX,
                                                op=ALU.add)
                    nc.vector.tensor_reduce(dsum[:, 2:3], dsum[:, 0:2],
                                            axis=AXX, op=ALU.add)
                    u_f = vp.tile([P, 1], f32, tag="uf", bufs=3,
                                  name=f"uf_{b}_{lt}")
                    u_b = vp.tile([P, 1], bf16, tag="ub", bufs=3,
                                  name=f"ub_{b}_{lt}")
                    nc.vector.reciprocal_approx_fast(out=u_f, in_=dsum[:, 2:3])
                    with nc.allow_low_precision(reason="bf16 softmax scale"):
                        nc.vector.tensor_copy(u_b, u_f)
                    pend_u[0] = (u_b, e2n_s, lt)
                return flush_u

            def make_tail(b, hTn_s, xT_s, x_s, rrow_ps, q2red_s,
                          do_q2mv=True, fink=None):
                """Item tail, split in three so it can be emitted interleaved
                into the next item's stage-1 engine streams."""
                st = {}

                def tail_a():
                    if do_q2mv:
                        with nc.allow_low_precision(reason="f32r sums"):
                            nc.vector.tensor_reduce(mv_s[:, D2T:C2T, b:b + 1],
                                                    q2red_s, axis=AXX,
                                                    op=ALU.add)
                    rflat_s = vp.tile([1, L], bf16, tag="rflat", bufs=1,
                                      name=f"rflat_{b}")
                    nc.scalar.copy(rflat_s[0:1, 0:LC], rrow_ps[0][0:1, :])
                    nc.scalar.copy(rflat_s[0:1, LC:L], rrow_ps[1][0:1, :])
                    # r row -> column chunks: K=1 matmuls into disjoint
                    # columns of one psum bank
                    rc_ps = pp.tile([P, LT], f32, tag="ps", name=f"rc_{b}")
                    for mt in range(LT):
                        mm(rc_ps[:, mt:mt + 1],
                           rflat_s[0:1, mt * P:(mt + 1) * P],
                           onesr_s[0:1, 0:1], mt == 0, mt == LT - 1)
                    rsum_s = vp.tile([P, LT], bf16, tag="rsum", bufs=1,
                                     name=f"rsum_{b}")
                    with nc.allow_low_precision(reason="bf16 r"):
                        nc.vector.tensor_copy(rsum_s, rc_ps)
                    st["rflat"] = rflat_s
                    st["rsum"] = rsum_s

                def tail_b():
                    rbc_s = vp.tile([P, L], bf16, tag="rbc", bufs=1,
                                    name=f"rbc_{b}")
                    for j in range(NLC):
                        ps_b = pp.tile([P, LC], f32, tag="ps")
                        mm(ps_b, onesr_s, st["rflat"][0:1, j * LC:(j + 1) * LC],
                           True, True)
                        with nc.allow_low_precision(reason="bf16 r"):
                            nc.vector.tensor_copy(rbc_s[:, j * LC:(j + 1) * LC],
                                                  ps_b)
                    st["rbc"] = rbc_s

                def tail_c():
                    rbc_s = st["rbc"]
                    rsum_s = st["rsum"]
                    with nc.allow_low_precision(reason="f32r rounding of sums"):
                        # x part: sum_m x[m,d] r[m] as tiny bf16 matmuls
                        for dt in range(DT):
                            ps_x = pp.tile([P, 1], f32, tag="ps",
                                           name=f"psx_{b}_{dt}")
                            for mk in range(LT):
                                mm(ps_x, x_s[:, mk, dt * P:(dt + 1) * P],
                                   rsum_s[:, mk:mk + 1], mk == 0, mk == LT - 1)
                            nc.vector.tensor_copy(mv_s[:, DT + dt, b:b + 1],
                                                  ps_x)
                            if fink:
                                fink(DT + dt, False)
                        # h part: transposed layout -> DVE weighted
                        # row-sums, split in halves for finer pipelining
                        hh = vp.tile([P, DT, 2], f32, tag="hh", bufs=1,
                                     name=f"hh_{b}")
                        for dt in range(DT):
                            for hf in range(2):
                                fs = slice(hf * LC, (hf + 1) * LC)
                                nc.vector.tensor_mul(hTn_s[:, dt, fs],
                                                     hTn_s[:, dt, fs], rbc_s[:, fs])
                                nc.vector.tensor_reduce(hh[:, dt, hf:hf + 1],
                                                        hTn_s[:, dt, fs],
                                                        axis=AXX, op=ALU.add)
                            nc.vector.tensor_reduce(mv_s[:, dt, b:b + 1],
                                                    hh[:, dt, :], axis=AXX,
                                                    op=ALU.add)
                            if fink:
                                fink(dt, dt == DT - 1)

                return tail_a, tail_b, tail_c

            pending = None
            nxt = None
            fin = {}
            nonlocal_state = {}
            for b in range(B):
                if nxt is None:
                    xT_s = xtp.tile([P, DT, L], f32r, tag="xT")
                    nc.sync.dma_start(out=w1t_s[:, 0:2, :],
                                      in_=w1t_ext[0:2 * P, :]
                                      .rearrange("(k p) e -> p k e", p=P))
                    nc.sync.dma_start(out=xT_s[:, 0:2, :],
                                      in_=xT_ext[b, 0:2 * P, :]
                                      .rearrange("(k p) l -> p k l", p=P))
                    nc.sync.dma_start(out=w1t_s[:, 2:DT, :],
                                      in_=w1t_ext[2 * P:DT * P, :]
                                      .rearrange("(k p) e -> p k e", p=P))
                    nc.sync.dma_start(out=xT_s[:, 2:DT, :],
                                      in_=xT_ext[b, 2 * P:DT * P, :]
                                      .rearrange("(k p) l -> p k l", p=P))
                    x_s = xp.tile([P, LT, D], bf16, tag="x")
                    nc.sync.dma_start(
                        out=x_s, in_=x_ext[b].rearrange("(k p) d -> p k d", p=P))
                    nc.sync.dma_start(
                        out=wo1t_s, in_=wo1t_ext.rearrange("(k p) e -> p k e", p=P))
                    nc.sync.dma_start(
                        out=w2t_s, in_=w2t_ext.rearrange("(k p) e -> p k e", p=P))
                    nc.sync.dma_start(
                        out=wo2t_s, in_=wo2t_ext.rearrange("(k p) e -> p k e", p=P))
                    qT0 = emit_ph1(xT_s, 0)
                else:
                    xT_s, x_s, qT0 = nxt
                    nxt = None
                hTn_s = hp.tile([P, DT, L], f32r, tag="hTn")
                q2red_s = vp.tile([P, D2T, NLC], f32r, tag="q2red", bufs=1,
                                  name=f"q2red_{b}")

                # ---- stage 1, lc0, with the previous item's tail
                # interleaved late enough that the PE stream has runway
                # before each tail matmul group
                exp0, psd0 = emit_ph2(xT_s, qT0, b, 0)
                bc1 = bc_recip(psd0)
                mix0 = emit_ph3(x_s, exp0, bc1, b, 0)
                if pending:
                    pending[0]()                    # r row extraction (PE+ACT)
                emit_ph4(mix0, qT0, hTn_s, 0)

                # ---- stage 1, lc1 (ph5 of lc0 slotted between PE phases)
                qT1 = emit_ph1(xT_s, 1)
                if pending:
                    pending[1]()                    # r broadcast (PE+ACT)
                exp1, psd1 = emit_ph2(xT_s, qT1, b, 1)
                emit_ph5(hTn_s, b, 0)
                bc1b = bc_recip(psd1)
                mix1 = emit_ph3(x_s, exp1, bc1b, b, 1)
                emit_ph4(mix1, qT1, hTn_s, 1)
                if pending:
                    pending[2]()                    # weighted row-sums (DVE)
                    pending = None

                # ---- stage 2 (ph5 of lc1 hidden behind ph7 of lc0;
                # ph7 of lc1 sliced into ph8(lc0)'s chain shadows; the next
                # item's DMA+ph1 (or the final q2-column matmuls) slice into
                # ph8(lc1)'s last chain shadow)
                rrow_ps = [pp.tile([P, LC], f32, tag="ps", name=f"rrow_{b}_{i}")
                           for i in range(NLC)]
                q2T0 = tp.tile([P, D2T, LC], f32r, tag="q2", name=f"q2_{b}_0")
                emit_ph7(hTn_s, xT_s, q2red_s, q2T0, 0, 0, 4)
                emit_ph5(hTn_s, b, 1)
                emit_ph7(hTn_s, xT_s, q2red_s, q2T0, 0, 4, D2T)
                pu0 = emit_ph8_nat(hTn_s, xT_s, q2T0, rrow_ps, b, 0)
                q2T1 = tp.tile([P, D2T, LC], f32r, tag="q2", name=f"q2_{b}_1")
                emit_ph7(hTn_s, xT_s, q2red_s, q2T1, 1, 0, 2)
                pu0()
                emit_ph7(hTn_s, xT_s, q2red_s, q2T1, 1, 2, D2T)

                if b < B - 1:
                    def head_next(bn=b + 1):
                        xTn = xtp.tile([P, DT, L], f32r, tag="xT")
                        nc.sync.dma_start(
                            out=xTn,
                            in_=xT_ext[bn].rearrange("(k p) l -> p k l", p=P))
                        xn = xp.tile([P, LT, D], bf16, tag="x")
                        nc.sync.dma_start(
                            out=xn,
                            in_=x_ext[bn].rearrange("(k p) d -> p k d", p=P))
                        nonlocal_state["nxt"] = (xTn, xn, emit_ph1(xTn, 0))
                    last_fill = head_next
                else:
                    def last_fill():
                        with nc.allow_low_precision(reason="f32r sums"):
                            nc.vector.tensor_reduce(mv_s[:, D2T:C2T, b:b + 1],
                                                    q2red_s, axis=AXX,
                                                    op=ALU.add)
                        emb_ps = pp.tile([P, LC], f32, tag="ps", name="emb_ps")
                        for i, ck in enumerate(range(D2T, C2T)):
                            mm(emb_ps[0:B, :], mv_s[:, ck, :], wo2t_s[:, ck, :],
                               i == 0, False)
                        fin["emb_ps"] = emb_ps
                pu1 = emit_ph8_nat(hTn_s, xT_s, q2T1, rrow_ps, b, 1)
                last_fill()
                pu1()
                if b < B - 1:
                    nxt = nonlocal_state.pop("nxt")

                def fink(ck, last, bb=b):
                    if bb == B - 1:
                        mm(fin["emb_ps"][0:B, :], mv_s[:, ck, :],
                           wo2t_s[:, ck, :], False, last)

                pending = make_tail(b, hTn_s, xT_s, x_s, rrow_ps, q2red_s,
                                    do_q2mv=(b < B - 1),
                                    fink=fink if b == B - 1 else None)

            # last item's tail (final Wo2 matmuls ride inside via fink)
            pending[0]()
            pending[1]()
            pending[2]()
            emb_ps = fin["emb_ps"]
            embf_s = vp.tile([B, D], f32, tag="bc", bufs=2, name="embf")
            nc.vector.tensor_copy(embf_s, emb_ps[0:B, :])
            nc.sync.dma_start(out=out_ext[:, :], in_=embf_s)

    _t1 = _time.time()
    nc.compile()
    print(f"[kernel] tile-trace+schedule {_t1 - _t0:.1f}s, "
          f"bacc compile {_time.time() - _t1:.1f}s", file=sys.stderr, flush=True)
    return nc


def get_nc():
    # the pipelined item tail reads xT(b) during item b+1, so the xT pool
    # MUST be double-buffered -- no xt_bufs=1 fallback (it deadlocks)
    if "nc" not in _CACHE:
        _CACHE["nc"] = _build_nc(xt_bufs=2)
    return _CACHE["nc"]


def make_in_maps(x, W1, Wo1, W2, Wo2):
    import ml_dtypes
    x = np.ascontiguousarray(np.asarray(x, dtype=np.float32))
    xT = np.ascontiguousarray(x.transpose(0, 2, 1))
    x_bf = np.ascontiguousarray(x.astype(ml_dtypes.bfloat16))
    w1t = np.ascontiguousarray(np.asarray(W1, np.float32).T)
    wo1t = np.ascontiguousarray(np.asarray(Wo1, np.float32).T)
    w2t = np.ascontiguousarray(np.asarray(W2, np.float32).T)
    # 1/L mean-scale folded into Wo2 (it only feeds the final matmuls)
    wo2t = np.ascontiguousarray((np.asarray(Wo2, np.float32).T / L).astype(ml_dtypes.bfloat16))
    onesm = np.ones((P, P), dtype=ml_dtypes.bfloat16)
    onesr = np.ones((1, P), dtype=ml_dtypes.bfloat16)
    return [
        {"x": x_bf[c * B:(c + 1) * B], "xT": xT[c * B:(c + 1) * B],
         "w1t": w1t, "wo1t": wo1t, "w2t": w2t, "wo2t": wo2t,
         "onesm": onesm, "onesr": onesr}
        for c in range(NCORES)
    ]


def run(x, W1, Wo1, W2, Wo2, trace=False, **kw):
    from concourse.bass_utils import run_bass_kernel_spmd
    nc = get_nc()
    in_maps = make_in_maps(x, W1, Wo1, W2, Wo2)
    res = run_bass_kernel_spmd(nc, in_maps, core_ids=list(range(NCORES)),
                               trace=trace, **kw)
    out = np.concatenate([res.results[c]["out"] for c in range(NCORES)], axis=0)
    return out.reshape(N_GLOBAL, D, 1, 1), res


def kernel(**inputs):
    out, _ = run(inputs["x"], inputs["W1"], inputs["Wo1"],
                 inputs["W2"], inputs["Wo2"])
    return out


# revision 33
# speedup vs baseline: 1.0227x; 1.0025x over previous
"""AttentionFuserV3 Trainium2 kernel: 8-core pure data parallel over batch.

Reference computation per batch item x_b [L=1024, D=512]:
  stage1: q = x W1^T; S = q x^T; A = softmax(S); mix = A x;
          h = tanh([mix, q] Wo1^T); h = h / max(||h||_2, eps)     (per row)
  stage2: c = [h, x]; q2 = c W2^T; S2 = q2 c^T; A2 = softmax(S2);
          mix2 = A2 c; o = [mix2, q2] Wo2^T; emb = mean_l(o)

Layout strategy ("T-space"): all big tensors are kept transposed in SBUF
(feature dim on partitions, sequence dim L on the free axis) so every
matmul contraction lands on the partition axis without on-device
transposes of the attention matrix.  Softmax runs without max-subtraction
(|scores| < ~70, exp stays in f32 range); softmax denominators are
accumulated pre-broadcast with a ones-matrix lhsT (M=128 costs the same
as M=1 on the PE) and inverted full-width with the fast approximate
reciprocal.

Stage-2 exploits linearity of the final mean:
  emb = mean_l(out2) = (1/L) [sum_l mix2 ; sum_l q2] Wo2^T
  sum_l mix2 = c2^T r   with   r[m] = sum_l exp2[m,l] / denom2[l]
so mix2 and out2 are never materialized per position; r is reduced on
DVE from the transposed exp2 tile, broadcast back with rank-1 matmuls,
and the Wo2 projection happens once for all batch items at the end.

The per-item tail (r extraction, broadcast, weighted row-sums) is
software-pipelined: its PE/DVE work is emitted interleaved into the NEXT
item's stage-1 stream so the PE never waits on the serial DVE chain.

Matmuls run in float32r (full PE speed at N=512); the attention
probabilities and the mix lhsT (x natural) are bf16.
"""

import sys

sys.path.insert(0, "/opt/trn_rl_repo")

import numpy as np

N_GLOBAL, L, D = 32, 1024, 512
NCORES = 8
B = N_GLOBAL // NCORES          # 4 batch items per core
P = 128
LC = 512                        # l-chunk (matmul moving free dim)
NLC = L // LC                   # 2
DT = D // P                     # 4
LT = L // P                     # 8
D2T = 2 * D // P                # 8
C2T = 4 * D // P                # 16

_CACHE = {}


def _build_nc(xt_bufs=2):
    import concourse.bass as bass  # noqa: F401
    import concourse.mybir as mybir
    import concourse.tile as tile
    from concourse import bacc

    f32 = mybir.dt.float32
    f32r = mybir.dt.float32r
    bf16 = mybir.dt.bfloat16
    AF = mybir.ActivationFunctionType
    ALU = mybir.AluOpType
    AXX = mybir.AxisListType.X

    nc = bacc.Bacc("TRN2", target_bir_lowering=False, debug=False,
                   num_devices=NCORES)

    x_ext = nc.declare_dram_parameter("x", [B, L, D], bf16, isOutput=False)
    xT_ext = nc.declare_dram_parameter("xT", [B, D, L], f32r, isOutput=False)
    w1t_ext = nc.declare_dram_parameter("w1t", [D, D], f32r, isOutput=False)
    wo1t_ext = nc.declare_dram_parameter("wo1t", [2 * D, D], f32r, isOutput=False)
    w2t_ext = nc.declare_dram_parameter("w2t", [2 * D, 2 * D], f32r, isOutput=False)
    wo2t_ext = nc.declare_dram_parameter("wo2t", [4 * D, D], bf16, isOutput=False)
    # Constants shipped from host: walrus's ISA check rejects memset/iota
    # writes into float32r tiles, but DMA from an f32r DRAM param is fine.
    onm_ext = nc.declare_dram_parameter("onesm", [P, P], bf16, isOutput=False)
    onr_ext = nc.declare_dram_parameter("onesr", [1, P], bf16, isOutput=False)
    out_ext = nc.declare_dram_parameter("out", [B, D], f32, isOutput=True)

    import time as _time
    _t0 = _time.time()
    with tile.TileContext(nc) as tc:
        with tc.tile_pool(name="wp", bufs=1) as wp, \
             tc.tile_pool(name="cp", bufs=1) as cp, \
             tc.tile_pool(name="xtp", bufs=xt_bufs) as xtp, \
             tc.tile_pool(name="xp", bufs=2) as xp, \
             tc.tile_pool(name="hp", bufs=2) as hp, \
             tc.tile_pool(name="tp", bufs=1) as tp, \
             tc.tile_pool(name="vp", bufs=2) as vp, \
             tc.tile_pool(name="ep", bufs=1) as ep, \
             tc.tile_pool(name="ps", bufs=8, space="PSUM") as pp:

            # ---- w1t gates ph1 of item 0: DMA it first (chunks of it
            # interleave with the xT chunks inside the b==0 iteration)
            w1t_s = wp.tile([P, DT, D], f32r, tag="w1t")
            onesm_s = cp.tile([P, P], bf16, tag="onesm")
            nc.sync.dma_start(out=onesm_s, in_=onm_ext[:, :])
            onesr_s = cp.tile([1, P], bf16, tag="onesr")
            nc.sync.dma_start(out=onesr_s, in_=onr_ext[:, :])

            wo1t_s = wp.tile([P, D2T, D], f32r, tag="wo1t")
            w2t_s = wp.tile([P, D2T, 2 * D], f32r, tag="w2t")
            wo2t_s = wp.tile([P, C2T, D], bf16, tag="wo2t")

            # meanvec columns for the deferred Wo2 projection:
            # c-chunks 0..3 = sum_l mix2 (h part), 4..7 (x part), 8..15 = sum_l q2
            mv_s = ep.tile([P, C2T, B], bf16, tag="mv")

            def mm(out, lhsT, rhs, first, last):
                nc.tensor.matmul(out, lhsT, rhs, start=first, stop=last)

            def bc_recip(denom_ps):
                """[128,512] PSUM pre-broadcast softmax denominator ->
                [128,512] SBUF approx reciprocal (values in (~1e-28, 1e33):
                safely inside approx_fast's domain)."""
                bc = vp.tile([P, LC], f32, tag="bc", bufs=2)
                nc.vector.reciprocal_approx_fast(out=bc, in_=denom_ps)
                return bc

            def emit_ph1(xT_s, lc):
                ls = slice(lc * LC, (lc + 1) * LC)
                qT_s = tp.tile([P, DT, LC], f32r, tag="qt")
                for et in range(DT):
                    ps = pp.tile([P, LC], f32, tag="ps")
                    for dk in range(DT):
                        mm(ps, w1t_s[:, dk, et * P:(et + 1) * P],
                           xT_s[:, dk, ls], dk == 0, dk == DT - 1)
                    # alternate engines so the copy backlog never delays
                    # ph2's first exp on ACT
                    if et % 2 == 0:
                        nc.scalar.copy(qT_s[:, et, :], ps)
                    else:
                        with nc.allow_low_precision(reason="f32r store"):
                            nc.vector.tensor_copy(qT_s[:, et, :], ps)
                return qT_s

            def emit_ph2(xT_s, qT_s, b, lc):
                expT_s = tp.tile([P, LT, LC], bf16, tag="exp",
                                 name=f"exp1_{b}_{lc}")
                ps_d = pp.tile([P, LC], f32, tag="ps")
                for mt in range(LT):
                    ps = pp.tile([P, LC], f32, tag="ps")
                    for ek in range(DT):
                        mm(ps, xT_s[:, ek, mt * P:(mt + 1) * P],
                           qT_s[:, ek, :], ek == 0, ek == DT - 1)
                    nc.scalar.activation(expT_s[:, mt, :], ps, AF.Exp)
                    mm(ps_d, onesm_s, expT_s[:, mt, :], mt == 0, mt == LT - 1)
                return expT_s, ps_d

            def emit_ph3(x_s, expT_s, bc1, b, lc):
                mixT_s = tp.tile([P, DT, LC], f32r, tag="mix",
                                 name=f"mix_{b}_{lc}")
                for dt in range(DT):
                    ps = pp.tile([P, LC], f32, tag="ps")
                    for mk in range(LT):
                        mm(ps, x_s[:, mk, dt * P:(dt + 1) * P],
                           expT_s[:, mk, :], mk == 0, mk == LT - 1)
                    nc.vector.tensor_mul(mixT_s[:, dt, :], ps, bc1)
                return mixT_s

            def emit_ph4(mixT_s, qT_s, hTn_s, lc):
                """out1 -> tanh, written unnormalized into hTn[:, :, ls];
                ck-outer so the first matmuls only need wo1t chunk 0."""
                ls = slice(lc * LC, (lc + 1) * LC)
                for ot in range(DT):
                    ps = pp.tile([P, LC], f32, tag="ps")
                    for ck in range(D2T):
                        rhs = mixT_s[:, ck, :] if ck < DT else qT_s[:, ck - DT, :]
                        mm(ps, wo1t_s[:, ck, ot * P:(ot + 1) * P],
                           rhs, ck == 0, ck == D2T - 1)
                    nc.scalar.activation(hTn_s[:, ot, ls], ps, AF.Tanh)

            def emit_ph5(hTn_s, b, lc):
                """L2-normalize hTn[:, :, ls] in place (norm over the
                partition axis via ones-matmul)."""
                ls = slice(lc * LC, (lc + 1) * LC)
                hsq_s = tp.tile([P, DT, LC], bf16, tag="mix", name=f"hsq_{b}_{lc}")
                for dt in range(DT):
                    nc.vector.tensor_mul(hsq_s[:, dt, :], hTn_s[:, dt, ls],
                                         hTn_s[:, dt, ls])
                ps_n = pp.tile([P, LC], f32, tag="ps")
                for dt in range(DT):
                    mm(ps_n, onesm_s, hsq_s[:, dt, :], dt == 0, dt == DT - 1)
                bcn = vp.tile([P, LC], f32, tag="bc", bufs=2, name=f"bcn_{b}_{lc}")
                bc2 = vp.tile([P, LC], f32, tag="bc", bufs=2, name=f"bc2_{b}_{lc}")
                nc.scalar.activation(bcn, ps_n, AF.Sqrt)
                nc.vector.tensor_scalar_max(bcn, bcn, 1e-12)
                nc.vector.reciprocal_approx_fast(out=bc2, in_=bcn)
                for dt in range(DT):
                    nc.vector.tensor_mul(hTn_s[:, dt, ls], hTn_s[:, dt, ls], bc2)

            def emit_ph7(hTn_s, xT_s, q2red_s, q2T_s, lc, et_lo, et_hi):
                ls = slice(lc * LC, (lc + 1) * LC)

                def c2T(k, fs):
                    return hTn_s[:, k, fs] if k < DT else xT_s[:, k - DT, fs]

                for et in range(et_lo, et_hi):
                    ps = pp.tile([P, LC], f32, tag="ps")
                    for dk in range(D2T):
                        mm(ps, w2t_s[:, dk, et * P:(et + 1) * P],
                           c2T(dk, ls), dk == 0, dk == D2T - 1)
                    nc.scalar.copy(q2T_s[:, et, :], ps)
                    # q2 column-sum partial, per et so it pipelines
                    # behind the copies instead of one monolithic reduce
                    with nc.allow_low_precision(reason="f32r rounding of sums"):
                        nc.vector.tensor_reduce(q2red_s[:, et, lc:lc + 1],
                                                q2T_s[:, et, :], axis=AXX,
                                                op=ALU.add)

            def emit_ph8_nat(hTn_s, xT_s, q2T_s, rrow_ps, b, lc):
                """Stage-2 attention in NATURAL orientation (query l on
                partitions): per l-tile, the softmax denominator is a free-
                axis DVE reduce and r accumulates via matmuls with the
                reciprocal vector as lhsT -- r = sum_lt u_lt^T @ exp2n_lt.
                No pre-broadcast denominator matmuls, no serial r block."""
                def c2T(k, fs):
                    return hTn_s[:, k, fs] if k < DT else xT_s[:, k - DT, fs]

                pend_u = [None]

                def flush_u():
                    if pend_u[0] is not None:
                        pu_b, pe2n, plt = pend_u[0]
                        pend_u[0] = None
                        for ms in range(NLC):
                            mm(rrow_ps[ms][0:1, :], pu_b, pe2n[:, ms, :],
                               plt == 0, plt == LT - 1)

                for li in range(LT // NLC):
                    lt = lc * (LT // NLC) + li
                    loff = li * P
                    e2n_s = tp.tile([P, NLC, LC], bf16, tag="e2n",
                                    name=f"e2n_{b}_{lt}", bufs=3)
                    dsum = vp.tile([P, 3], f32, tag="dsum", bufs=3,
                                   name=f"dsum_{b}_{lt}")
                    ps2 = [pp.tile([P, LC], f32, tag="ps",
                                   name=f"ps8_{b}_{lt}_{i}") for i in range(NLC)]
                    for ek in range(D2T):
                        for ms in range(NLC):
                            mm(ps2[ms], q2T_s[:, ek, loff:loff + P],
                               c2T(ek, slice(ms * LC, (ms + 1) * LC)),
                               ek == 0, ek == D2T - 1)
                    flush_u()   # previous lt's u-matmuls, now chain-covered
                    for ms in range(NLC):
                        nc.scalar.activation(e2n_s[:, ms, :], ps2[ms], AF.Exp)
                        nc.vector.tensor_reduce(dsum[:, ms:ms + 1],
                                                e2n_s[:, ms, :], axis=AXX,
                                                op=ALU.add)
                    nc.vector.tensor_reduce(dsum[:, 2:3], dsum[:, 0:2],
                                            axis=AXX, op=ALU.add)
                    u_f = vp.tile([P, 1], f32, tag="uf", bufs=3,
                                  name=f"uf_{b}_{lt}")
                    u_b = vp.tile([P, 1], bf16, tag="ub", bufs=3,
                                  name=f"ub_{b}_{lt}")
                    nc.vector.reciprocal_approx_fast(out=u_f, in_=dsum[:, 2:3])
                    with nc.allow_low_precision(reason="bf16 softmax scale"):
                        nc.vector.tensor_copy(u_b, u_f)
                    pend_u[0] = (u_b, e2n_s, lt)
                return flush_u

            def make_tail(b, hTn_s, xT_s, x_s, rrow_ps, q2red_s,
                          do_q2mv=True, fink=None):
                """Item tail, split in three so it can be emitted interleaved
                into the next item's stage-1 engine streams."""
                st = {}

                def tail_a():
                    if do_q2mv:
                        with nc.allow_low_precision(reason="f32r sums"):
                            nc.vector.tensor_reduce(mv_s[:, D2T:C2T, b:b + 1],
                                                    q2red_s, axis=AXX,
                                                    op=ALU.add)
                    rflat_s = vp.tile([1, L], bf16, tag="rflat", bufs=1,
                                      name=f"rflat_{b}")
                    nc.scalar.copy(rflat_s[0:1, 0:LC], rrow_ps[0][0:1, :])
                    nc.scalar.copy(rflat_s[0:1, LC:L], rrow_ps[1][0:1, :])
                    # r row -> column chunks: K=1 matmuls into disjoint
                    # columns of one psum bank
                    rc_ps = pp.tile([P, LT], f32, tag="ps", name=f"rc_{b}")
                    for mt in range(LT):
                        mm(rc_ps[:, mt:mt + 1],
                           rflat_s[0:1, mt * P:(mt + 1) * P],
                           onesr_s[0:1, 0:1], mt == 0, mt == LT - 1)
                    rsum_s = vp.tile([P, LT], bf16, tag="rsum", bufs=1,
                                     name=f"rsum_{b}")
                    with nc.allow_low_precision(reason="bf16 r"):
                        nc.vector.tensor_copy(rsum_s, rc_ps)
                    st["rflat"] = rflat_s
                    st["rsum"] = rsum_s

                def tail_b():
                    rbc_s = vp.tile([P, L], bf16, tag="rbc", bufs=1,
                                    name=f"rbc_{b}")
                    for j in range(NLC):
                        ps_b = pp.tile([P, LC], f32, tag="ps")
                        mm(ps_b, onesr_s, st["rflat"][0:1, j * LC:(j + 1) * LC],
                           True, True)
                        with nc.allow_low_precision(reason="bf16 r"):
                            nc.vector.tensor_copy(rbc_s[:, j * LC:(j + 1) * LC],
                                                  ps_b)
                    st["rbc"] = rbc_s

                def tail_c():
                    rbc_s = st["rbc"]
                    rsum_s = st["rsum"]
                    with nc.allow_low_precision(reason="f32r rounding of sums"):
                        # x part: sum_m x[m,d] r[m] as tiny bf16 matmuls
                        for dt in range(DT):
                            ps_x = pp.tile([P, 1], f32, tag="ps",
                                           name=f"psx_{b}_{dt}")
                            for mk in range(LT):
                                mm(ps_x, x_s[:, mk, dt * P:(dt + 1) * P],
                                   rsum_s[:, mk:mk + 1], mk == 0, mk == LT - 1)
                            nc.vector.tensor_copy(mv_s[:, DT + dt, b:b + 1],
                                                  ps_x)
                            if fink:
                                fink(DT + dt, False)
                        # h part: transposed layout -> DVE weighted
                        # row-sums, split in halves for finer pipelining
                        hh = vp.tile([P, DT, 2], f32, tag="hh", bufs=1,
                                     name=f"hh_{b}")
                        for dt in range(DT):
                            for hf in range(2):
                                fs = slice(hf * LC, (hf + 1) * LC)
                                nc.vector.tensor_mul(hTn_s[:, dt, fs],
                                                     hTn_s[:, dt, fs], rbc_s[:, fs])
                                nc.vector.tensor_reduce(hh[:, dt, hf:hf + 1],
                                                        hTn_s[:, dt, fs],
                                                        axis=AXX, op=ALU.add)
                            nc.vector.tensor_reduce(mv_s[:, dt, b:b + 1],
                                                    hh[:, dt, :], axis=AXX,
                                                    op=ALU.add)
                            if fink:
                                fink(dt, dt == DT - 1)

                return tail_a, tail_b, tail_c

            pending = None
            nxt = None
            fin = {}
            nonlocal_state = {}
            for b in range(B):
                if nxt is None:
                    xT_s = xtp.tile([P, DT, L], f32r, tag="xT")
                    nc.sync.dma_start(out=w1t_s[:, 0:2, :],
                                      in_=w1t_ext[0:2 * P, :]
                                      .rearrange("(k p) e -> p k e", p=P))
                    nc.sync.dma_start(out=xT_s[:, 0:2, :],
                                      in_=xT_ext[b, 0:2 * P, :]
                                      .rearrange("(k p) l -> p k l", p=P))
                    nc.sync.dma_start(out=w1t_s[:, 2:DT, :],
                                      in_=w1t_ext[2 * P:DT * P, :]
                                      .rearrange("(k p) e -> p k e", p=P))
                    nc.sync.dma_start(out=xT_s[:, 2:DT, :],
                                      in_=xT_ext[b, 2 * P:DT * P, :]
                                      .rearrange("(k p) l -> p k l", p=P))
                    x_s = xp.tile([P, LT, D], bf16, tag="x")
                    nc.sync.dma_start(
                        out=x_s, in_=x_ext[b].rearrange("(k p) d -> p k d", p=P))
                    nc.sync.dma_start(
                        out=wo1t_s, in_=wo1t_ext.rearrange("(k p) e -> p k e", p=P))
                    nc.sync.dma_start(
                        out=w2t_s, in_=w2t_ext.rearrange("(k p) e -> p k e", p=P))
                    nc.sync.dma_start(
                        out=wo2t_s, in_=wo2t_ext.rearrange("(k p) e -> p k e", p=P))
                    qT0 = emit_ph1(xT_s, 0)
                else:
                    xT_s, x_s, qT0 = nxt
                    nxt = None
                hTn_s = hp.tile([P, DT, L], f32r, tag="hTn")
                q2red_s = vp.tile([P, D2T, NLC], f32r, tag="q2red", bufs=1,
                                  name=f"q2red_{b}")

                # ---- stage 1, lc0, with the previous item's tail
                # interleaved late enough that the PE stream has runway
                # before each tail matmul group
                exp0, psd0 = emit_ph2(xT_s, qT0, b, 0)
                bc1 = bc_recip(psd0)
                mix0 = emit_ph3(x_s, exp0, bc1, b, 0)
                if pending:
                    pending[0]()                    # r row extraction (PE+ACT)
                emit_ph4(mix0, qT0, hTn_s, 0)

                # ---- stage 1, lc1 (ph5 of lc0 slotted between PE phases)
                qT1 = emit_ph1(xT_s, 1)
                if pending:
                    pending[1]()                    # r broadcast (PE+ACT)
                exp1, psd1 = emit_ph2(xT_s, qT1, b, 1)
                emit_ph5(hTn_s, b, 0)
                bc1b = bc_recip(psd1)
                mix1 = emit_ph3(x_s, exp1, bc1b, b, 1)
                emit_ph4(mix1, qT1, hTn_s, 1)
                if pending:
                    pending[2]()                    # weighted row-sums (DVE)
                    pending = None

                # ---- stage 2 (ph5 of lc1 hidden behind ph7 of lc0;
                # ph7 of lc1 sliced into ph8(lc0)'s chain shadows; the next
                # item's DMA+ph1 (or the final q2-column matmuls) slice into
                # ph8(lc1)'s last chain shadow)
                rrow_ps = [pp.tile([P, LC], f32, tag="ps", name=f"rrow_{b}_{i}")
                           for i in range(NLC)]
                q2T0 = tp.tile([P, D2T, LC], f32r, tag="q2", name=f"q2_{b}_0")
                emit_ph7(hTn_s, xT_s, q2red_s, q2T0, 0, 0, 4)
                emit_ph5(hTn_s, b, 1)
                emit_ph7(hTn_s, xT_s, q2red_s, q2T0, 0, 4, D2T)
                pu0 = emit_ph8_nat(hTn_s, xT_s, q2T0, rrow_ps, b, 0)
                q2T1 = tp.tile([P, D2T, LC], f32r, tag="q2", name=f"q2_{b}_1")
                emit_ph7(hTn_s, xT_s, q2red_s, q2T1, 1, 0, 2)
                pu0()
                emit_ph7(hTn_s, xT_s, q2red_s, q2T1, 1, 2, D2T)

                if b < B - 1:
                    def head_next(bn=b + 1):
                        xTn = xtp.tile([P, DT, L], f32r, tag="xT")
                        nc.sync.dma_start(
                            out=xTn,
                            in_=xT_ext[bn].rearrange("(k p) l -> p k l", p=P))
                        xn = xp.tile([P, LT, D], bf16, tag="x")
                        nc.sync.dma_start(
                            out=xn,
                            in_=x_ext[bn].rearrange("(k p) d -> p k d", p=P))
                        nonlocal_state["nxt"] = (xTn, xn, emit_ph1(xTn, 0))
                    last_fill = head_next
                else:
                    def last_fill():
                        with nc.allow_low_precision(reason="f32r sums"):
                            nc.vector.tensor_reduce(mv_s[:, D2T:C2T, b:b + 1],
                                                    q2red_s, axis=AXX,
                                                    op=ALU.add)
                        emb_ps = pp.tile([P, LC], f32, tag="ps", name="emb_ps")
                        for i, ck in enumerate(range(D2T, C2T)):
                            mm(emb_ps[0:B, :], mv_s[:, ck, :], wo2t_s[:, ck, :],
                               i == 0, False)
                        fin["emb_ps"] = emb_ps
                pu1 = emit_ph8_nat(hTn_s, xT_s, q2T1, rrow_ps, b, 1)
                last_fill()
                pu1()
                if b < B - 1:
                    nxt = nonlocal_state.pop("nxt")

                def fink(ck, last, bb=b):
                    if bb == B - 1:
                        mm(fin["emb_ps"][0:B, :], mv_s[:, ck, :],
                           wo2t_s[:, ck, :], False, last)

                pending = make_tail(b, hTn_s, xT_s, x_s, rrow_ps, q2red_s,
                                    do_q2mv=(b < B - 1),
                                    fink=fink if b == B - 1 else None)

            # last item's tail (final Wo2 matmuls ride inside via fink)
            pending[0]()
            pending[1]()
            pending[2]()
            emb_ps = fin["emb_ps"]
            embf_s = vp.tile([B, D], f32, tag="bc", bufs=2, name="embf")
            nc.vector.tensor_copy(embf_s, emb_ps[0:B, :])
            nc.sync.dma_start(out=out_ext[:, :], in_=embf_s)

    _t1 = _time.time()
    nc.compile()
    print(f"[kernel] tile-trace+schedule {_t1 - _t0:.1f}s, "
          f"bacc compile {_time.time() - _t1:.1f}s", file=sys.stderr, flush=True)
    return nc


def get_nc():
    # the pipelined item tail reads xT(b) during item b+1, so the xT pool
    # MUST be double-buffered -- no xt_bufs=1 fallback (it deadlocks)
    if "nc" not in _CACHE:
        _CACHE["nc"] = _build_nc(xt_bufs=2)
    return _CACHE["nc"]


def make_in_maps(x, W1, Wo1, W2, Wo2):
    import ml_dtypes
    x = np.ascontiguousarray(np.asarray(x, dtype=np.float32))
    xT = np.ascontiguousarray(x.transpose(0, 2, 1))
    x_bf = np.ascontiguousarray(x.astype(ml_dtypes.bfloat16))
    w1t = np.ascontiguousarray(np.asarray(W1, np.float32).T)
    wo1t = np.ascontiguousarray(np.asarray(Wo1, np.float32).T)
    w2t = np.ascontiguousarray(np.asarray(W2, np.float32).T)
    # 1/L mean-scale folded into Wo2 (it only feeds the final matmuls)
    wo2t = np.ascontiguousarray((np.asarray(Wo2, np.float32).T / L).astype(ml_dtypes.bfloat16))
    onesm = np.ones((P, P), dtype=ml_dtypes.bfloat16)
    onesr = np.ones((1, P), dtype=ml_dtypes.bfloat16)
    return [
        {"x": x_bf[c * B:(c + 1) * B], "xT": xT[c * B:(c + 1) * B],
         "w1t": w1t, "wo1t": wo1t, "w2t": w2t, "wo2t": wo2t,
         "onesm": onesm, "onesr": onesr}
        for c in range(NCORES)
    ]


def run(x, W1, Wo1, W2, Wo2, trace=False, **kw):
    from concourse.bass_utils import run_bass_kernel_spmd
    nc = get_nc()
    in_maps = make_in_maps(x, W1, Wo1, W2, Wo2)
    res = run_bass_kernel_spmd(nc, in_maps, core_ids=list(range(NCORES)),
                               trace=trace, **kw)
    out = np.concatenate([res.results[c]["out"] for c in range(NCORES)], axis=0)
    return out.reshape(N_GLOBAL, D, 1, 1), res


def kernel(**inputs):
    out, _ = run(inputs["x"], inputs["W1"], inputs["Wo1"],
                 inputs["W2"], inputs["Wo2"])
    return out
